# revision 1
# baseline (speedup 1.0000x reference)
"""ChebyshevSheafFilter Trainium2 kernel (8 NeuronCores, edge/row-parallel).

Self-contained: host-side preprocessing (numpy) + Bass/Tile SPMD kernel via
run_bass_kernel_spmd. Takes FULL inputs, returns FULL output.

Math (matches reference):
    degree = bincount(row) + bincount(col)
    L(x)   = (2/(lam+1e-8)) * (degree*x - segsum_row(Q[e] @ x[col[e]])) - x
    T0=h, T1=L(h), Tk = 2 L(T_{k-1}) - T_{k-2}
    out = sum_k softmax(coeffs)[k] * Tk

Sharding: edges sorted by destination row; core c owns node rows
[c*npc,(c+1)*npc). Per Chebyshev step each core computes its own node slice
of T_next, then an 8-core AllGather rebuilds the full node table (gather
source for the next step's x[col]).

Device pipeline per step (per core):
    GPS dma_gather: xc = x[col] from a 256B-padded node table (4 int16-indexed
        sub-tables; only the first 16 floats of each gathered slot are read)
    DVE:  prod[e,(j,i)] = Qt[e,(j,i)] * xc[e,j]      (stride-0 broadcast AP)
    DVE:  S[e,c] = (row_local[e] == c)               (one-hot scatter matrix)
    PE :  psum[c,(j,i)] += S^T @ prod                (segment-sum, PSUM accum)
    DVE:  off[c,i] = sum_j psum[c,(j,i)]             (strided reduce)
    DVE:  T_next = a*(deg*x - off) - x [ *2 - T_prev ];  r += w_k*T_next
    CC :  AllGather(padded T_next slice) -> full padded table

Edge layout: per core, edges are bucketed by (dest 128-node group g, source
sub-table class t), each (g,t) segment padded to a block multiple of 128
(uniform across cores so the SPMD instruction stream is identical; per-core
variation lives in the data).
"""

import math
import os

import numpy as np

import concourse.bacc as bacc
import concourse.bass as bass
import concourse.mybir as mybir
import concourse.tile as tile
from concourse.bass_utils import run_bass_kernel_spmd

N_CORES = 8
D = 16
ORDER = 4
P = 128
ES = 64  # padded table slot size (floats) = 256B, dma_gather elem granularity
N_CLS = 4  # sub-table classes (int16 index range)
SG = 6  # groups per gather super-group

# ---------------------------------------------------------------------------
# walrus workaround: this build rejects instructions with >1 semaphore wait
# ("Too many sync wait commands"). Spill overflow waits onto dedicated nops
# placed just before the offender on the same engine.
_spill_counter = [0]


def _spill_excess_waits(nc, max_waits=1):
    n_spilled = 0
    for fn in nc.m.functions:
        for bb in fn.blocks:
            insts = list(bb.instructions)
            new_list = []
            changed = False
            for inst in insts:
                si = inst.sync_info
                waits = list(si.on_wait) if si is not None and si.on_wait else []
                if len(waits) > max_waits:
                    changed = True
                    keep = waits[-max_waits:]
                    overflow = waits[: len(waits) - max_waits]
                    for i in range(0, len(overflow), max_waits):
                        chunk = overflow[i : i + max_waits]
                        _spill_counter[0] += 1
                        nop = mybir.InstNoOp(
                            name=f"waitspill-{_spill_counter[0]}",
                            engine=inst.engine,
                            bass_nofuse=True,
                            sync_info=mybir.SyncInfo(on_wait=chunk, on_update=[]),
                        )
                        new_list.append(nop)
                        n_spilled += 1
                    si.on_wait = keep
                new_list.append(inst)
            if changed:
                live = bb.instructions
                live.clear()
                live.extend(new_list)
    return n_spilled


# ---------------------------------------------------------------------------
# host preprocessing


def _table_idx(n, npc, n_groups):
    """node id -> slot in the permuted node table layout."""
    c = n // npc
    r = n % npc
    p = r % P
    g = r // P
    return (c * P + p) * n_groups + g


def _wrap_idx16(flat):
    """Wrap a flat int16 index stream into dma_gather's [128, n/16] layout
    (16-partition interleave, replicated 8x down the partitions)."""
    n = flat.shape[0]
    assert n % 16 == 0
    w = np.zeros((16, n // 16), dtype=np.int16)
    for p in range(16):
        w[p] = flat[p::16]
    return np.tile(w, (8, 1))


def _preprocess(h, Q, coeffs, edge_index, lambda_max):
    N = h.shape[0]
    E = edge_index.shape[1]
    assert N % N_CORES == 0
    npc = N // N_CORES
    n_groups = math.ceil(npc / P)
    slots = n_groups * P  # node slots per core
    table_rows = N_CORES * slots
    subt = (table_rows + N_CLS - 1) // N_CLS  # sub-table slot count
    assert subt <= 32767, "int16 sub-table index overflow"

    row = np.asarray(edge_index[0], dtype=np.int64)
    col = np.asarray(edge_index[1], dtype=np.int64)
    h = np.asarray(h, dtype=np.float32)
    Q = np.asarray(Q, dtype=np.float32)

    lam = float(np.asarray(lambda_max, dtype=np.float64))
    alpha = 2.0 / (lam + 1e-8)
    cf = np.asarray(coeffs, dtype=np.float64)
    w = np.exp(cf - cf.max())
    w = w / w.sum()

    degree = (
        np.bincount(row, minlength=N) + np.bincount(col, minlength=N)
    ).astype(np.float32)

    col_tab_all = _table_idx(col, npc, n_groups)
    cls_all = col_tab_all // subt  # gather class per edge

    # sort edges by (core, group, class) — host-side bucketing
    core_all = row // npc
    g_all = (row % npc) // P
    key = ((core_all * n_groups) + g_all) * N_CLS + cls_all
    perm = np.argsort(key, kind="stable")
    key_s = key[perm]
    row_s = row[perm]
    col_tab = col_tab_all[perm]
    cls_s = cls_all[perm]

    n_seg = N_CORES * n_groups * N_CLS
    sizes = np.bincount(key_s, minlength=n_seg).reshape(N_CORES, n_groups * N_CLS)
    # uniform block count per (g,t) across cores
    NB_gt = ((sizes.max(axis=0) + P - 1) // P).astype(np.int64)
    NB_gt = NB_gt.reshape(n_groups, N_CLS)
    # each group needs >=1 block so its PSUM accumulation group exists
    empty_g = NB_gt.sum(axis=1) == 0
    NB_gt[empty_g, 0] = 1
    NB_g = NB_gt.sum(axis=1)  # [n_groups]
    gt_base = np.zeros(n_groups * N_CLS, dtype=np.int64)
    gt_base[1:] = np.cumsum(NB_gt.reshape(-1))[:-1]
    gt_base = gt_base.reshape(n_groups, N_CLS)
    g_base = np.zeros(n_groups + 1, dtype=np.int64)
    g_base[1:] = np.cumsum(NB_g)
    nblk = int(g_base[-1])

    # per-edge placement: rank within (core, g, t) segment
    seg_start = np.zeros(n_seg, dtype=np.int64)
    np.cumsum(sizes.reshape(-1)[:-1], out=seg_start[1:])
    rank = np.arange(E, dtype=np.int64) - seg_start[key_s]
    p_e = rank % P
    gt_of_edge = key_s % (n_groups * N_CLS)
    blk_e = gt_base.reshape(-1)[gt_of_edge] + rank // P

    rowl_e = ((row_s % npc) % P).astype(np.float32)

    # class-stream prefix: pos_t[t, g] = class-t blocks before group g
    pos_t = np.zeros((N_CLS, n_groups + 1), dtype=np.int64)
    for t in range(N_CLS):
        pos_t[t, 1:] = np.cumsum(NB_gt[:, t])
    nblk_t = pos_t[:, -1]

    # padded node table of h (step-1 gather source)
    h_pad = np.zeros((subt * N_CLS, ES), dtype=np.float32)
    h_pad[_table_idx(np.arange(N, dtype=np.int64), npc, n_groups), :D] = h

    deg_pad = np.zeros((N_CORES, slots), dtype=np.float32)
    deg_pad[:, :npc] = degree.reshape(N_CORES, npc)
    deg_t = np.ascontiguousarray(
        deg_pad.reshape(N_CORES, n_groups, P).transpose(0, 2, 1)
    )

    iota = np.tile(np.arange(P, dtype=np.float32), (P, 1))

    in_maps = []
    for c in range(N_CORES):
        m = (key_s // (n_groups * N_CLS)) == c
        e_idx = perm[m]
        p_c = p_e[m]
        blk_c = blk_e[m]
        t_c = cls_s[m]
        gt_c = gt_of_edge[m]
        g_c = gt_c // N_CLS

        Qs = np.zeros((nblk * P, D * D), dtype=np.float32)
        Qs[blk_c * P + p_c] = Q[e_idx].transpose(0, 2, 1).reshape(-1, D * D)

        rowl_t = np.zeros((P, nblk), dtype=np.float32)
        rowl_t[p_c, blk_c] = rowl_e[m]

        # per-class index streams (position = class-stream block * 128 + p)
        idx_cols = []
        for t in range(N_CLS):
            stream = np.zeros(max(int(nblk_t[t]), 0) * P, dtype=np.int16)
            mt = t_c == t
            gs = g_c[mt]
            pos_in_stream = (
                pos_t[t][gs] + (blk_c[mt] - gt_base[gs, t])
            ) * P + p_c[mt]
            stream[pos_in_stream] = (col_tab[m][mt] - t * subt).astype(np.int16)
            if stream.shape[0]:
                idx_cols.append(_wrap_idx16(stream))
        idx_arr = np.concatenate(idx_cols, axis=1)
        assert idx_arr.shape == (P, nblk * 8)

        x0 = np.zeros((P, n_groups * D), dtype=np.float32)
        x0r = x0.reshape(P, n_groups, D)
        hslice = h.reshape(N_CORES, npc, D)[c]
        full_g = npc // P
        x0r[:, :full_g] = (
            hslice[: full_g * P].reshape(full_g, P, D).transpose(1, 0, 2)
        )
        if npc % P:
            x0r[: npc % P, full_g] = hslice[full_g * P :]

        in_maps.append(
            {
                "qs": Qs,
                "rowl_t": rowl_t,
                "idx_in": idx_arr,
                "deg_t": deg_t[c],
                "h_pad": h_pad,
                "iota": iota,
                "x0": x0,
            }
        )

    meta = dict(
        N=N,
        npc=npc,
        n_groups=n_groups,
        slots=slots,
        table_rows=table_rows,
        subt=subt,
        nblk=nblk,
        NB_gt=NB_gt,
        NB_g=NB_g,
        gt_base=gt_base,
        g_base=g_base,
        pos_t=pos_t,
        nblk_t=nblk_t,
        alpha=alpha,
        w=[float(x) for x in w],
    )
    return in_maps, meta


# ---------------------------------------------------------------------------
# device program


def _build_nc(meta):
    n_groups = meta["n_groups"]
    nblk = meta["nblk"]
    subt = meta["subt"]
    NB_gt = meta["NB_gt"]
    NB_g = meta["NB_g"]
    gt_base = meta["gt_base"]
    g_base = meta["g_base"]
    pos_t = meta["pos_t"]
    nblk_t = meta["nblk_t"]
    alpha = meta["alpha"]
    w = meta["w"]
    GF = n_groups * D
    max_nbg = int(NB_g.max())
    max_nbgt = int(NB_gt.max())
    n_sg = (n_groups + SG - 1) // SG
    max_sg_t = 1
    for sgi in range(n_sg):
        g0, g1 = sgi * SG, min((sgi + 1) * SG, n_groups)
        for t in range(N_CLS):
            max_sg_t = max(max_sg_t, int(pos_t[t, g1] - pos_t[t, g0]))

    # class offsets into idx_in (in columns; 8 columns per block)
    cls_col = np.zeros(N_CLS + 1, dtype=np.int64)
    cls_col[1:] = np.cumsum(nblk_t) * 8

    f32 = mybir.dt.float32
    i16 = mybir.dt.int16
    nc = bacc.Bacc(None)

    qs = nc.declare_dram_parameter("qs", [nblk * P, D * D], f32, isOutput=False)
    rowl_in = nc.declare_dram_parameter("rowl_t", [P, nblk], f32, isOutput=False)
    idx_in = nc.declare_dram_parameter("idx_in", [P, nblk * 8], i16, isOutput=False)
    deg_in = nc.declare_dram_parameter("deg_t", [P, n_groups], f32, isOutput=False)
    h_pad = nc.declare_dram_parameter(
        "h_pad", [subt * N_CLS, ES], f32, isOutput=False
    )
    iota_in = nc.declare_dram_parameter("iota", [P, P], f32, isOutput=False)
    x0_in = nc.declare_dram_parameter("x0", [P, GF], f32, isOutput=False)
    r_out = nc.declare_dram_parameter("r_out", [P, GF], f32, isOutput=True)

    ag_in = [
        nc.dram_tensor(f"ag_in{k}", [P * n_groups, ES], f32) for k in (1, 2, 3)
    ]
    ag_out = [
        nc.dram_tensor(f"ag_out{k}", [subt * N_CLS, ES], f32, addr_space="Shared")
        for k in (1, 2, 3)
    ]

    with tile.TileContext(nc) as tc:
        with (
            tc.tile_pool(name="const", bufs=1) as const_pool,
            tc.tile_pool(name="state", bufs=1) as state_pool,
            tc.tile_pool(name="qg", bufs=2) as q_pool,
            tc.tile_pool(name="stage", bufs=2) as stage_pool,
            tc.tile_pool(name="idx", bufs=3) as idx_pool,
            tc.tile_pool(name="prod", bufs=3) as prod_pool,
            tc.tile_pool(name="sgen", bufs=3) as s_pool,
            tc.tile_pool(name="off", bufs=2) as off_pool,
            tc.tile_pool(name="upd", bufs=2) as upd_pool,
            tc.tile_pool(name="psum", bufs=4, space="PSUM") as psum_pool,
        ):
            rowl_t = const_pool.tile([P, nblk], f32)
            deg_t = const_pool.tile([P, n_groups], f32)
            iota_t = const_pool.tile([P, P], f32)
            nc.sync.dma_start(out=rowl_t[:], in_=rowl_in[:])
            nc.sync.dma_start(out=deg_t[:], in_=deg_in[:])
            nc.sync.dma_start(out=iota_t[:], in_=iota_in[:])

            xa = state_pool.tile([P, GF], f32, tag="xa")
            xb = state_pool.tile([P, GF], f32, tag="xb")
            r_t = state_pool.tile([P, GF], f32, tag="r")
            nc.sync.dma_start(out=xa[:], in_=x0_in[:])
            x, tp = xa, xb

            cur_table = h_pad
            for k in range(1, ORDER + 1):
                wk = w[k]
                off_full = off_pool.tile([P, GF], f32, tag="off")
                for sgi in range(n_sg):
                    g0, g1 = sgi * SG, min((sgi + 1) * SG, n_groups)
                    stage = [None] * N_CLS
                    for t in range(N_CLS):
                        nb = int(pos_t[t, g1] - pos_t[t, g0])
                        if nb == 0:
                            continue
                        ncols = nb * 8
                        it = idx_pool.tile([P, max_sg_t * 8], i16, tag="idx")
                        nc.sync.dma_start(
                            out=it[:, :ncols],
                            in_=idx_in[
                                :,
                                int(cls_col[t] + pos_t[t, g0] * 8) : int(
                                    cls_col[t] + pos_t[t, g1] * 8
                                ),
                            ],
                        )
                        st = stage_pool.tile(
                            [P, max_sg_t * ES], f32, tag=f"stage{t}"
                        )
                        # chunk calls to <=8 blocks (1024 idxs): the SWDGE
                        # descriptor ring holds ~1024 descriptors
                        for b0 in range(0, nb, 8):
                            bn = min(8, nb - b0)
                            nc.gpsimd.dma_gather(
                                out_ap=st[
                                    :, b0 * ES : (b0 + bn) * ES
                                ].rearrange("p (b e) -> p b e", e=ES),
                                in_ap=cur_table[t * subt : (t + 1) * subt, :],
                                idxs_ap=it[:, b0 * 8 : (b0 + bn) * 8],
                                num_idxs=bn * P,
                                num_idxs_reg=bn * P,
                                elem_size=ES,
                            )
                        stage[t] = st
                    for g in range(g0, g1):
                        nbg = int(NB_g[g])
                        qg = q_pool.tile([P, max_nbg * D * D], f32, tag="qg")
                        nc.sync.dma_start(
                            out=qg[:, : nbg * D * D],
                            in_=qs[:]
                            .rearrange("(b p) f -> p b f", p=P)[
                                :, int(g_base[g]) : int(g_base[g + 1]), :
                            ],
                        )
                        acc = psum_pool.tile([P, D * D], f32, tag="acc")
                        for t in range(N_CLS):
                            nb = int(NB_gt[g, t])
                            if nb == 0:
                                continue
                            segoff = int(gt_base[g, t] - g_base[g])
                            stoff = int(pos_t[t, g] - pos_t[t, g0])
                            prod = prod_pool.tile(
                                [P, max_nbgt * D * D], f32, tag="prod"
                            )
                            nc.vector.tensor_tensor(
                                out=prod[:, : nb * D * D].rearrange(
                                    "p (b j i) -> p b j i", b=nb, j=D
                                ),
                                in0=qg[
                                    :, segoff * D * D : (segoff + nb) * D * D
                                ].rearrange("p (b j i) -> p b j i", b=nb, j=D),
                                in1=stage[t][:, stoff * ES : (stoff + nb) * ES]
                                .rearrange("p (b e) -> p b e", e=ES)[:, :, :D]
                                .unsqueeze(3)
                                .to_broadcast([P, nb, D, D]),
                                op=mybir.AluOpType.mult,
                            )
                            S8 = s_pool.tile([P, max_nbgt * P], f32, tag="S8")
                            nc.vector.tensor_tensor(
                                out=S8[:, : nb * P].rearrange(
                                    "p (b c) -> p b c", b=nb
                                ),
                                in0=rowl_t[
                                    :,
                                    int(gt_base[g, t]) : int(gt_base[g, t]) + nb,
                                ]
                                .unsqueeze(2)
                                .to_broadcast([P, nb, P]),
                                in1=iota_t[:]
                                .unsqueeze(1)
                                .to_broadcast([P, nb, P]),
                                op=mybir.AluOpType.is_equal,
                            )
                            for b in range(nb):
                                blk = int(gt_base[g, t]) + b
                                nc.tensor.matmul(
                                    out=acc[:],
                                    lhsT=S8[:, b * P : (b + 1) * P],
                                    rhs=prod[:, b * D * D : (b + 1) * D * D],
                                    start=(blk == int(g_base[g])),
                                    stop=(blk == int(g_base[g + 1]) - 1),
                                )
                        nc.vector.tensor_reduce(
                            out=off_full[:, g * D : (g + 1) * D],
                            in_=acc[:].rearrange("p (j i) -> p i j", j=D),
                            axis=mybir.AxisListType.X,
                            op=mybir.AluOpType.add,
                        )

                # ---- node update (whole slice at once) ----
                tmp = upd_pool.tile([P, GF], f32, tag="tmp")
                nc.vector.tensor_tensor(
                    out=tmp[:].rearrange("p (g i) -> p g i", g=n_groups),
                    in0=x[:].rearrange("p (g i) -> p g i", g=n_groups),
                    in1=deg_t[:].unsqueeze(2).to_broadcast([P, n_groups, D]),
                    op=mybir.AluOpType.mult,
                )
                nc.vector.tensor_tensor(
                    out=tmp[:],
                    in0=tmp[:],
                    in1=off_full[:],
                    op=mybir.AluOpType.subtract,
                )
                nc.vector.tensor_scalar_mul(tmp[:], tmp[:], alpha)
                tmp2 = upd_pool.tile([P, GF], f32, tag="tmp2")
                if k == 1:
                    nc.vector.tensor_tensor(
                        out=tp[:], in0=tmp[:], in1=x[:], op=mybir.AluOpType.subtract
                    )
                    nc.vector.tensor_scalar_mul(r_t[:], x[:], w[0])
                    nc.vector.tensor_scalar_mul(tmp2[:], tp[:], wk)
                    nc.vector.tensor_tensor(
                        out=r_t[:], in0=r_t[:], in1=tmp2[:], op=mybir.AluOpType.add
                    )
                else:
                    nc.vector.tensor_tensor(
                        out=tmp[:], in0=tmp[:], in1=x[:], op=mybir.AluOpType.subtract
                    )
                    nc.vector.tensor_scalar_mul(tmp[:], tmp[:], 2.0)
                    nc.vector.tensor_tensor(
                        out=tp[:], in0=tmp[:], in1=tp[:], op=mybir.AluOpType.subtract
                    )
                    nc.vector.tensor_scalar_mul(tmp2[:], tp[:], wk)
                    nc.vector.tensor_tensor(
                        out=r_t[:], in0=r_t[:], in1=tmp2[:], op=mybir.AluOpType.add
                    )
                x, tp = tp, x  # x now holds T_k

                if k < ORDER:
                    dst = ag_in[k - 1]
                    nc.sync.dma_start(
                        out=dst[:].rearrange("(p g) e -> p g e", p=P)[:, :, :D],
                        in_=x[:].rearrange("p (g i) -> p g i", g=n_groups),
                    )
                    nc.gpsimd.collective_compute(
                        "AllGather",
                        mybir.AluOpType.bypass,
                        ins=[dst[:]],
                        outs=[ag_out[k - 1][:]],
                        replica_groups=[list(range(N_CORES))],
                    )
                    cur_table = ag_out[k - 1]

            nc.sync.dma_start(out=r_out[:], in_=r_t[:])

    nc.compile()
    return nc


# ---------------------------------------------------------------------------

_CACHE = {}
LAST_RESULTS = None


def kernel(h, Q, coeffs, edge_index, lambda_max):
    h = np.asarray(h)
    Q = np.asarray(Q)
    coeffs = np.asarray(coeffs)
    edge_index = np.asarray(edge_index)
    lambda_max = np.asarray(lambda_max)

    import time as _time

    _t0 = _time.time()
    in_maps, meta = _preprocess(h, Q, coeffs, edge_index, lambda_max)
    _t1 = _time.time()

    key = (h.shape, Q.shape, edge_index.shape, meta["nblk"])
    if key not in _CACHE:
        nc = _build_nc(meta)
        _spill_excess_waits(nc)
        _CACHE[key] = nc
    nc = _CACHE[key]
    _t2 = _time.time()
    print(f"[kernel] preprocess {_t1-_t0:.1f}s  build+compile {_t2-_t1:.1f}s  nblk={meta['nblk']}", flush=True)

    trace = os.environ.get("CHEB_TRACE") == "1"
    res = run_bass_kernel_spmd(nc, in_maps, list(range(N_CORES)), trace=trace)
    print(f"[kernel] run {_time.time()-_t2:.1f}s", flush=True)
    global LAST_RESULTS
    LAST_RESULTS = res

    npc = meta["npc"]
    n_groups = meta["n_groups"]
    slots = meta["slots"]
    out = np.empty((h.shape[0], D), dtype=np.float32)
    for c in range(N_CORES):
        r = res.results[c]["r_out"]
        r3 = r.reshape(P, n_groups, D).transpose(1, 0, 2).reshape(slots, D)
        out[c * npc : (c + 1) * npc] = r3[:npc]
    return out



# revision 18
# speedup vs baseline: 2.2740x; 2.2740x over previous
"""ChebyshevSheafFilter Trainium2 kernel (8 NeuronCores, edge/row-parallel).

Self-contained: host-side preprocessing (numpy) + Bass/Tile SPMD kernel via
run_bass_kernel_spmd. Takes FULL inputs, returns FULL output.

Math (matches reference):
    degree = bincount(row) + bincount(col)
    L(x)   = (2/(lam+1e-8)) * (degree*x - segsum_row(Q[e] @ x[col[e]])) - x
    T0=h, T1=L(h), Tk = 2 L(T_{k-1}) - T_{k-2}
    out = sum_k softmax(coeffs)[k] * Tk

Sharding: edges sorted by destination row; core c owns node rows
[c*npc,(c+1)*npc). Per Chebyshev step each core computes its own node slice
of T_next, then an 8-core AllGather rebuilds the full node table (gather
source for the next step's x[col]).

Device pipeline per step (per core):
    GPS dma_gather (prepare_only + trigger): xc = x[col] from a 256B-padded
        node table (4 int16-indexed sub-tables); step 1 instead reads a
        host-precomputed edge-aligned xc1 (bf16, no gather needed)
    DVE:  xcb = bf16(xc[.., :16])                     (cast for 2x matmul)
    DVE:  prod[e,(j,i)] = Qb[e,(j,i)] * xcb[e,j]      (bf16, stride-0 bcast)
    DVE:  S[e,c] = (row_local[e] == c)                (bf16 one-hot)
    PE :  psum[c,(j,i)] += S^T @ prod                 (bf16 MM, f32 PSUM)
    DVE:  off[c,i] = sum_j psum[c,(j,i)]              (strided reduce, f32)
    DVE:  T_next = a*(deg*x - off) - x [ *2 - T_prev ];  r += w_k*T_next
    CC :  AllGather(padded T_next slice) -> full padded table

Edge layout: per core, edges are bucketed by (dest 128-node group g, source
sub-table class t), each (g,t) segment padded to a block multiple of 128
(uniform across cores so the SPMD instruction stream is identical; per-core
variation lives in the data). Q is staged partition-major ([128, nblk*256]
bf16) so per-group loads are 128 contiguous runs of nbg*512B.
"""

import math
import os

import ml_dtypes
import numpy as np

import concourse.bacc as bacc
import concourse.bass as bass
import concourse.mybir as mybir
import concourse.tile as tile
from concourse.bass_utils import run_bass_kernel_spmd

N_CORES = 8
D = 16
ORDER = 4
P = 128
ES = 64  # padded table slot size (floats) = 256B, dma_gather elem granularity
N_CLS = 4  # sub-table classes (int16 index range)
SG = 6  # groups per gather super-group
PREP_GATHER = False  # prepare_only+trigger_dma gathers (False: blocking)

BF16 = ml_dtypes.bfloat16

# ---------------------------------------------------------------------------
# walrus workaround: this build rejects instructions with >1 semaphore wait
# ("Too many sync wait commands"). Spill overflow waits onto dedicated nops
# placed just before the offender on the same engine.
_spill_counter = [0]


def _spill_excess_waits(nc, max_waits=1):
    n_spilled = 0
    for fn in nc.m.functions:
        for bb in fn.blocks:
            insts = list(bb.instructions)
            new_list = []
            changed = False
            for inst in insts:
                si = inst.sync_info
                waits = list(si.on_wait) if si is not None and si.on_wait else []
                if len(waits) > max_waits:
                    changed = True
                    keep = waits[-max_waits:]
                    overflow = waits[: len(waits) - max_waits]
                    for i in range(0, len(overflow), max_waits):
                        chunk = overflow[i : i + max_waits]
                        _spill_counter[0] += 1
                        nop = mybir.InstNoOp(
                            name=f"waitspill-{_spill_counter[0]}",
                            engine=inst.engine,
                            bass_nofuse=True,
                            sync_info=mybir.SyncInfo(on_wait=chunk, on_update=[]),
                        )
                        new_list.append(nop)
                        n_spilled += 1
                    si.on_wait = keep
                new_list.append(inst)
            if changed:
                live = bb.instructions
                live.clear()
                live.extend(new_list)
    return n_spilled


# ---------------------------------------------------------------------------
# host preprocessing


def _table_idx(n, npc, n_groups):
    """node id -> slot in the permuted node table layout."""
    c = n // npc
    r = n % npc
    p = r % P
    g = r // P
    return (c * P + p) * n_groups + g


def _wrap_idx16(flat):
    """Wrap a flat int16 index stream into dma_gather's [128, n/16] layout
    (16-partition interleave, replicated 8x down the partitions)."""
    n = flat.shape[0]
    assert n % 16 == 0
    w = np.zeros((16, n // 16), dtype=np.int16)
    for p in range(16):
        w[p] = flat[p::16]
    return np.tile(w, (8, 1))


def _preprocess(h, Q, coeffs, edge_index, lambda_max):
    N = h.shape[0]
    E = edge_index.shape[1]
    assert N % N_CORES == 0
    npc = N // N_CORES
    n_groups = math.ceil(npc / P)
    slots = n_groups * P  # node slots per core
    table_rows = N_CORES * slots
    subt = (table_rows + N_CLS - 1) // N_CLS  # sub-table slot count
    assert subt <= 32767, "int16 sub-table index overflow"

    row = np.asarray(edge_index[0], dtype=np.int64)
    col = np.asarray(edge_index[1], dtype=np.int64)
    h = np.asarray(h, dtype=np.float32)
    Q = np.asarray(Q, dtype=np.float32)

    lam = float(np.asarray(lambda_max, dtype=np.float64))
    alpha = 2.0 / (lam + 1e-8)
    cf = np.asarray(coeffs, dtype=np.float64)
    w = np.exp(cf - cf.max())
    w = w / w.sum()

    degree = (
        np.bincount(row, minlength=N) + np.bincount(col, minlength=N)
    ).astype(np.float32)

    col_tab_all = _table_idx(col, npc, n_groups)
    cls_all = col_tab_all // subt  # gather class per edge

    # sort edges by (core, group, class) — host-side bucketing
    core_all = row // npc
    g_all = (row % npc) // P
    key = ((core_all * n_groups) + g_all) * N_CLS + cls_all
    perm = np.argsort(key, kind="stable")
    key_s = key[perm]
    row_s = row[perm]
    col_s = col[perm]
    col_tab = col_tab_all[perm]
    cls_s = cls_all[perm]

    n_seg = N_CORES * n_groups * N_CLS
    sizes = np.bincount(key_s, minlength=n_seg).reshape(N_CORES, n_groups * N_CLS)
    # uniform block count per (g,t) across cores
    NB_gt = ((sizes.max(axis=0) + P - 1) // P).astype(np.int64)
    NB_gt = NB_gt.reshape(n_groups, N_CLS)
    # each group needs >=1 block so its PSUM accumulation group exists
    empty_g = NB_gt.sum(axis=1) == 0
    NB_gt[empty_g, 0] = 1
    NB_g = NB_gt.sum(axis=1)  # [n_groups]
    gt_base = np.zeros(n_groups * N_CLS, dtype=np.int64)
    gt_base[1:] = np.cumsum(NB_gt.reshape(-1))[:-1]
    gt_base = gt_base.reshape(n_groups, N_CLS)
    g_base = np.zeros(n_groups + 1, dtype=np.int64)
    g_base[1:] = np.cumsum(NB_g)
    nblk = int(g_base[-1])

    # per-edge placement: rank within (core, g, t) segment
    seg_start = np.zeros(n_seg, dtype=np.int64)
    np.cumsum(sizes.reshape(-1)[:-1], out=seg_start[1:])
    rank = np.arange(E, dtype=np.int64) - seg_start[key_s]
    p_e = rank % P
    gt_of_edge = key_s % (n_groups * N_CLS)
    blk_e = gt_base.reshape(-1)[gt_of_edge] + rank // P

    rowl_e = ((row_s % npc) % P).astype(np.float32)

    # class-stream prefix: pos_t[t, g] = class-t blocks before group g
    pos_t = np.zeros((N_CLS, n_groups + 1), dtype=np.int64)
    for t in range(N_CLS):
        pos_t[t, 1:] = np.cumsum(NB_gt[:, t])
    nblk_t = pos_t[:, -1]

    deg_pad = np.zeros((N_CORES, slots), dtype=np.float32)
    deg_pad[:, :npc] = degree.reshape(N_CORES, npc)
    deg_t = np.ascontiguousarray(
        deg_pad.reshape(N_CORES, n_groups, P).transpose(0, 2, 1)
    )

    iota = np.tile(np.arange(P, dtype=np.float32), (P, 1)).astype(BF16)

    in_maps = []
    for c in range(N_CORES):
        m = (key_s // (n_groups * N_CLS)) == c
        e_idx = perm[m]
        p_c = p_e[m]
        blk_c = blk_e[m]
        t_c = cls_s[m]
        gt_c = gt_of_edge[m]
        g_c = gt_c // N_CLS

        # Q staged partition-major bf16: qs_t[p, blk*DD : (blk+1)*DD]
        Qs = np.zeros((P, nblk, D * D), dtype=BF16)
        Qs[p_c, blk_c] = Q[e_idx].transpose(0, 2, 1).reshape(-1, D * D).astype(BF16)
        Qs = Qs.reshape(P, nblk * D * D)

        # step-1 gather precomputed on host: xc1[p, blk*D:(blk+1)*D] = h[col]
        xc1 = np.zeros((P, nblk, D), dtype=BF16)
        xc1[p_c, blk_c] = h[col_s[m]].astype(BF16)
        xc1 = xc1.reshape(P, nblk * D)

        rowl_t = np.zeros((P, nblk), dtype=BF16)
        rowl_t[p_c, blk_c] = rowl_e[m].astype(BF16)

        # per-class index streams (position = class-stream block * 128 + p)
        idx_cols = []
        for t in range(N_CLS):
            stream = np.zeros(max(int(nblk_t[t]), 0) * P, dtype=np.int16)
            mt = t_c == t
            gs = g_c[mt]
            pos_in_stream = (
                pos_t[t][gs] + (blk_c[mt] - gt_base[gs, t])
            ) * P + p_c[mt]
            stream[pos_in_stream] = (col_tab[m][mt] - t * subt).astype(np.int16)
            if stream.shape[0]:
                idx_cols.append(_wrap_idx16(stream))
        idx_arr = np.concatenate(idx_cols, axis=1)
        assert idx_arr.shape == (P, nblk * 8)

        x0 = np.zeros((P, n_groups * D), dtype=np.float32)
        x0r = x0.reshape(P, n_groups, D)
        hslice = h.reshape(N_CORES, npc, D)[c]
        full_g = npc // P
        x0r[:, :full_g] = (
            hslice[: full_g * P].reshape(full_g, P, D).transpose(1, 0, 2)
        )
        if npc % P:
            x0r[: npc % P, full_g] = hslice[full_g * P :]

        in_maps.append(
            {
                "qs": Qs,
                "xc1": xc1,
                "rowl_t": rowl_t,
                "idx_in": idx_arr,
                "deg_t": deg_t[c],
                "iota": iota,
                "x0": x0,
            }
        )

    meta = dict(
        N=N,
        npc=npc,
        n_groups=n_groups,
        slots=slots,
        table_rows=table_rows,
        subt=subt,
        nblk=nblk,
        NB_gt=NB_gt,
        NB_g=NB_g,
        gt_base=gt_base,
        g_base=g_base,
        pos_t=pos_t,
        nblk_t=nblk_t,
        alpha=alpha,
        w=[float(x) for x in w],
    )
    return in_maps, meta


# ---------------------------------------------------------------------------
# device program


def _build_nc(meta, detect_races=True):
    n_groups = meta["n_groups"]
    nblk = meta["nblk"]
    subt = meta["subt"]
    NB_gt = meta["NB_gt"]
    NB_g = meta["NB_g"]
    gt_base = meta["gt_base"]
    g_base = meta["g_base"]
    pos_t = meta["pos_t"]
    nblk_t = meta["nblk_t"]
    alpha = meta["alpha"]
    w = meta["w"]
    GF = n_groups * D
    max_nbg = int(NB_g.max())
    max_nbgt = int(NB_gt.max())
    n_sg = (n_groups + SG - 1) // SG
    max_sg_t = 1
    for sgi in range(n_sg):
        g0, g1 = sgi * SG, min((sgi + 1) * SG, n_groups)
        for t in range(N_CLS):
            max_sg_t = max(max_sg_t, int(pos_t[t, g1] - pos_t[t, g0]))

    # class offsets into idx_in (in columns; 8 columns per block)
    cls_col = np.zeros(N_CLS + 1, dtype=np.int64)
    cls_col[1:] = np.cumsum(nblk_t) * 8

    f32 = mybir.dt.float32
    bf16 = mybir.dt.bfloat16
    i16 = mybir.dt.int16
    nc = bacc.Bacc(None, detect_race_conditions=detect_races)

    qs = nc.declare_dram_parameter("qs", [P, nblk * D * D], bf16, isOutput=False)
    xc1_in = nc.declare_dram_parameter("xc1", [P, nblk * D], bf16, isOutput=False)
    rowl_in = nc.declare_dram_parameter("rowl_t", [P, nblk], bf16, isOutput=False)
    idx_in = nc.declare_dram_parameter("idx_in", [P, nblk * 8], i16, isOutput=False)
    deg_in = nc.declare_dram_parameter("deg_t", [P, n_groups], f32, isOutput=False)
    iota_in = nc.declare_dram_parameter("iota", [P, P], bf16, isOutput=False)
    x0_in = nc.declare_dram_parameter("x0", [P, GF], f32, isOutput=False)
    r_out = nc.declare_dram_parameter("r_out", [P, GF], f32, isOutput=True)

    ag_in = [
        nc.dram_tensor(f"ag_in{k}", [P * n_groups, ES], f32) for k in (1, 2, 3)
    ]
    ag_out = [
        nc.dram_tensor(f"ag_out{k}", [subt * N_CLS, ES], f32, addr_space="Shared")
        for k in (1, 2, 3)
    ]

    with tile.TileContext(nc) as tc:
        # NOTE: must be allocated inside TileContext — the context resets the
        # free-semaphore pool at entry, so an earlier allocation would be
        # handed out again to Tile's internal DMA lanes (races, NaNs).
        dma_sem = nc.alloc_semaphore("swdge_gather")
        with (
            tc.tile_pool(name="const", bufs=1) as const_pool,
            tc.tile_pool(name="state", bufs=1) as state_pool,
            tc.tile_pool(name="qg", bufs=3) as q_pool,
            tc.tile_pool(name="stage", bufs=2) as stage_pool,
            tc.tile_pool(name="idx", bufs=1) as idx_pool,
            tc.tile_pool(name="xcb", bufs=2) as xcb_pool,
            tc.tile_pool(name="prod", bufs=3) as prod_pool,
            tc.tile_pool(name="qh", bufs=3) as qh_pool,
            tc.tile_pool(name="sgen", bufs=3) as s_pool,
            tc.tile_pool(name="off", bufs=2) as off_pool,
            tc.tile_pool(name="canary", bufs=1) as canary_pool,
            tc.tile_pool(name="upd", bufs=2) as upd_pool,
            tc.tile_pool(name="psum", bufs=4, space="PSUM") as psum_pool,
        ):
            rowl_t = const_pool.tile([P, nblk], bf16)
            deg_t = const_pool.tile([P, n_groups], f32)
            iota_t = const_pool.tile([P, P], bf16)
            idx_t = idx_pool.tile([P, nblk * 8], i16)
            xcb1 = const_pool.tile([P, nblk * D], bf16)
            nc.sync.dma_start(out=rowl_t[:], in_=rowl_in[:])
            nc.sync.dma_start(out=deg_t[:], in_=deg_in[:])
            nc.sync.dma_start(out=iota_t[:], in_=iota_in[:])
            nc.sync.dma_start(out=idx_t[:], in_=idx_in[:])
            nc.sync.dma_start(out=xcb1[:], in_=xc1_in[:])

            xa = state_pool.tile([P, GF], f32, tag="xa")
            xb = state_pool.tile([P, GF], f32, tag="xb")
            r_t = state_pool.tile([P, GF], f32, tag="r")
            nc.sync.dma_start(out=xa[:], in_=x0_in[:])
            x, tp = xa, xb
            prep_count = [0]  # cumulative prepare_only gathers (16 incs each)

            for k in range(1, ORDER + 1):
                wk = w[k]
                cur_table = ag_out[k - 2] if k >= 2 else None
                off_full = off_pool.tile([P, GF], f32, tag="off")
                for sgi in range(n_sg):
                    g0, g1 = sgi * SG, min((sgi + 1) * SG, n_groups)
                    xcb_sg = [None] * N_CLS
                    for t in range(N_CLS):
                        nb = int(pos_t[t, g1] - pos_t[t, g0])
                        if nb == 0:
                            continue
                        if k == 1:
                            # step 1: host-precomputed gather (xcb1) is read
                            # directly by the prod op below; nothing to stage.
                            continue
                        xcb = xcb_pool.tile(
                            [P, max_sg_t * D], bf16, tag=f"xcb{t}"
                        )
                        st = stage_pool.tile(
                            [P, max_sg_t * ES], f32, tag=f"stage{t}"
                        )
                        # chunk calls to <=8 blocks (1024 idxs): the SWDGE
                        # descriptor ring holds ~1024 descriptors
                        for b0 in range(0, nb, 8):
                            bn = min(8, nb - b0)
                            c0 = int(cls_col[t] + (pos_t[t, g0] + b0) * 8)
                            kw = (
                                dict(prepare_only=True, sem=dma_sem)
                                if PREP_GATHER
                                else {}
                            )
                            nc.gpsimd.dma_gather(
                                out_ap=st[
                                    :, b0 * ES : (b0 + bn) * ES
                                ].rearrange("p (b e) -> p b e", e=ES),
                                in_ap=cur_table[
                                    t * subt : (t + 1) * subt, :
                                ],
                                idxs_ap=idx_t[:, c0 : c0 + bn * 8],
                                num_idxs=bn * P,
                                num_idxs_reg=bn * P,
                                elem_size=ES,
                                **kw,
                            )
                            if PREP_GATHER:
                                prep_count[0] += 1
                        if PREP_GATHER:
                            nc.gpsimd.trigger_dma(count=None)
                        cast = nc.vector.tensor_scalar_mul(
                            xcb[:, : nb * D].rearrange(
                                "p (b j) -> p b j", j=D
                            ),
                            st[:, : nb * ES]
                            .rearrange("p (b e) -> p b e", e=ES)[:, :, :D],
                            1.0,
                        )
                        if PREP_GATHER:
                            # manual RAW guard: Tile's deferred-write gating
                            # of prep-mode gather consumers is unreliable;
                            # enforce DMA completion on the reader directly.
                            cast._wait_ge(dma_sem, 16 * prep_count[0])
                        xcb_sg[t] = xcb
                    for g in range(g0, g1):
                        nbg = int(NB_g[g])
                        qg = q_pool.tile([P, max_nbg * D * D], bf16, tag="qg")
                        nc.sync.dma_start(
                            out=qg[:, : nbg * D * D],
                            in_=qs[
                                :,
                                int(g_base[g])
                                * D
                                * D : int(g_base[g + 1])
                                * D
                                * D,
                            ],
                        )
                        acc = psum_pool.tile([P, D], f32, tag="acc")
                        for t in range(N_CLS):
                            nb = int(NB_gt[g, t])
                            if nb == 0:
                                continue
                            segoff = int(gt_base[g, t] - g_base[g])
                            stoff = int(pos_t[t, g] - pos_t[t, g0])
                            if k == 1:
                                xc_ap = xcb1[
                                    :,
                                    int(gt_base[g, t])
                                    * D : (int(gt_base[g, t]) + nb)
                                    * D,
                                ]
                            else:
                                xc_ap = xcb_sg[t][
                                    :, stoff * D : (stoff + nb) * D
                                ]
                            prod = prod_pool.tile(
                                [P, max_nbgt * D * D], bf16, tag="prod"
                            )
                            nc.vector.tensor_tensor(
                                out=prod[:, : nb * D * D].rearrange(
                                    "p (b j i) -> p b j i", b=nb, j=D
                                ),
                                in0=qg[
                                    :, segoff * D * D : (segoff + nb) * D * D
                                ].rearrange("p (b j i) -> p b j i", b=nb, j=D),
                                in1=xc_ap
                                .rearrange("p (b j) -> p b j", j=D)
                                .unsqueeze(3)
                                .to_broadcast([P, nb, D, D]),
                                op=mybir.AluOpType.mult,
                            )
                            qh = qh_pool.tile(
                                [P, max_nbgt * D], bf16, tag="qh"
                            )
                            # bf16 j-accumulation: validated 2.5e-3 rel err
                            # at full scale vs the 2e-2 tolerance
                            with nc.allow_low_precision(
                                reason="bf16 16-term j-sum, 8x under tol"
                            ):
                                nc.vector.tensor_reduce(
                                    out=qh[:, : nb * D].rearrange(
                                        "p (b i) -> p b i", i=D
                                    ),
                                    in_=prod[:, : nb * D * D].rearrange(
                                        "p (b j i) -> p b i j", b=nb, j=D
                                    ),
                                    axis=mybir.AxisListType.X,
                                    op=mybir.AluOpType.add,
                                )
                            S8 = s_pool.tile([P, max_nbgt * P], bf16, tag="S8")
                            nc.vector.tensor_tensor(
                                out=S8[:, : nb * P].rearrange(
                                    "p (b c) -> p b c", b=nb
                                ),
                                in0=rowl_t[
                                    :,
                                    int(gt_base[g, t]) : int(gt_base[g, t]) + nb,
                                ]
                                .unsqueeze(2)
                                .to_broadcast([P, nb, P]),
                                in1=iota_t[:]
                                .unsqueeze(1)
                                .to_broadcast([P, nb, P]),
                                op=mybir.AluOpType.is_equal,
                            )
                            for b in range(nb):
                                blk = int(gt_base[g, t]) + b
                                nc.tensor.matmul(
                                    out=acc[:],
                                    lhsT=S8[:, b * P : (b + 1) * P],
                                    rhs=qh[:, b * D : (b + 1) * D],
                                    start=(blk == int(g_base[g])),
                                    stop=(blk == int(g_base[g + 1]) - 1),
                                )
                        nc.scalar.copy(
                            out=off_full[:, g * D : (g + 1) * D],
                            in_=acc[:],
                        )

                # ---- node update (whole slice at once) ----
                tmp = upd_pool.tile([P, GF], f32, tag="tmp")
                nc.vector.tensor_tensor(
                    out=tmp[:].rearrange("p (g i) -> p g i", g=n_groups),
                    in0=x[:].rearrange("p (g i) -> p g i", g=n_groups),
                    in1=deg_t[:].unsqueeze(2).to_broadcast([P, n_groups, D]),
                    op=mybir.AluOpType.mult,
                )
                nc.vector.tensor_tensor(
                    out=tmp[:],
                    in0=tmp[:],
                    in1=off_full[:],
                    op=mybir.AluOpType.subtract,
                )
                nc.vector.tensor_scalar_mul(tmp[:], tmp[:], alpha)
                tmp2 = upd_pool.tile([P, GF], f32, tag="tmp2")
                if k == 1:
                    nc.vector.tensor_tensor(
                        out=tp[:], in0=tmp[:], in1=x[:], op=mybir.AluOpType.subtract
                    )
                    nc.vector.tensor_scalar_mul(r_t[:], x[:], w[0])
                    nc.vector.tensor_scalar_mul(tmp2[:], tp[:], wk)
                    nc.vector.tensor_tensor(
                        out=r_t[:], in0=r_t[:], in1=tmp2[:], op=mybir.AluOpType.add
                    )
                else:
                    nc.vector.tensor_tensor(
                        out=tmp[:], in0=tmp[:], in1=x[:], op=mybir.AluOpType.subtract
                    )
                    nc.vector.tensor_scalar_mul(tmp[:], tmp[:], 2.0)
                    nc.vector.tensor_tensor(
                        out=tp[:], in0=tmp[:], in1=tp[:], op=mybir.AluOpType.subtract
                    )
                    nc.vector.tensor_scalar_mul(tmp2[:], tp[:], wk)
                    nc.vector.tensor_tensor(
                        out=r_t[:], in0=r_t[:], in1=tmp2[:], op=mybir.AluOpType.add
                    )
                x, tp = tp, x  # x now holds T_k

                if k < ORDER:
                    dst = ag_in[k - 1]
                    nc.sync.dma_start(
                        out=dst[:].rearrange("(p g) e -> p g e", p=P)[:, :, :D],
                        in_=x[:].rearrange("p (g i) -> p g i", g=n_groups),
                    )
                    nc.gpsimd.collective_compute(
                        "AllGather",
                        mybir.AluOpType.bypass,
                        ins=[dst[:]],
                        outs=[ag_out[k - 1][:]],
                        replica_groups=[list(range(N_CORES))],
                    )
                    # Canary: blocking Pool-engine read of the AG output.
                    # Tile resolves gen-mode-0 collective->reader deps
                    # correctly (waits for ncfw completion); Pool in-order
                    # execution then fences the next step's trigger_dma
                    # calls behind AG completion. The deferred-dep path
                    # (trigger waiting on the collective directly) fires
                    # too early on HW.
                    canary = canary_pool.tile([P, 2], f32, tag="canary")
                    nc.gpsimd.dma_start(
                        out=canary[:], in_=ag_out[k - 1][0:P, 0:2]
                    )

            nc.sync.dma_start(out=r_out[:], in_=r_t[:])

    nc.compile()
    return nc


# ---------------------------------------------------------------------------

_CACHE = {}
LAST_RESULTS = None


def kernel(h, Q, coeffs, edge_index, lambda_max):
    h = np.asarray(h)
    Q = np.asarray(Q)
    coeffs = np.asarray(coeffs)
    edge_index = np.asarray(edge_index)
    lambda_max = np.asarray(lambda_max)

    import time as _time

    _t0 = _time.time()
    in_maps, meta = _preprocess(h, Q, coeffs, edge_index, lambda_max)
    _t1 = _time.time()

    key = (h.shape, Q.shape, edge_index.shape, meta["nblk"])
    if key not in _CACHE:
        nc = _build_nc(meta)
        _spill_excess_waits(nc)
        _CACHE[key] = nc
    nc = _CACHE[key]
    _t2 = _time.time()
    print(f"[kernel] preprocess {_t1-_t0:.1f}s  build+compile {_t2-_t1:.1f}s  nblk={meta['nblk']}", flush=True)

    trace = os.environ.get("CHEB_TRACE") == "1"
    res = run_bass_kernel_spmd(nc, in_maps, list(range(N_CORES)), trace=trace)
    print(f"[kernel] run {_time.time()-_t2:.1f}s", flush=True)
    global LAST_RESULTS
    LAST_RESULTS = res

    npc = meta["npc"]
    n_groups = meta["n_groups"]
    slots = meta["slots"]
    out = np.empty((h.shape[0], D), dtype=np.float32)
    for c in range(N_CORES):
        r = res.results[c]["r_out"]
        r3 = r.reshape(P, n_groups, D).transpose(1, 0, 2).reshape(slots, D)
        out[c * npc : (c + 1) * npc] = r3[:npc]
    return out


# revision 32
# speedup vs baseline: 3.1014x; 1.3639x over previous
"""ChebyshevSheafFilter Trainium2 kernel (8 NeuronCores, edge/row-parallel).

Self-contained: host-side preprocessing (numpy) + Bass/Tile SPMD kernel via
run_bass_kernel_spmd. Takes FULL inputs, returns FULL output.

Math (matches reference):
    degree = bincount(row) + bincount(col)
    L(x)   = (2/(lam+1e-8)) * (degree*x - segsum_row(Q[e] @ x[col[e]])) - x
    T0=h, T1=L(h), Tk = 2 L(T_{k-1}) - T_{k-2}
    out = sum_k softmax(coeffs)[k] * Tk

Sharding: edges sorted by destination row; core c owns node rows
[c*npc,(c+1)*npc). Per Chebyshev step each core computes its own node slice
of T_next, then an 8-core AllGather rebuilds the full node table (gather
source for the next step's x[col]).

Device pipeline per step (per core):
    GPS dma_gather (prepare_only + trigger): xc = x[col] from a 256B-padded
        node table (4 int16-indexed sub-tables); step 1 instead reads a
        host-precomputed edge-aligned xc1 (bf16, no gather needed)
    DVE:  xcb = bf16(xc[.., :16])                     (cast for 2x matmul)
    DVE:  prod[e,(j,i)] = Qb[e,(j,i)] * xcb[e,j]      (bf16, stride-0 bcast)
    DVE:  S[e,c] = (row_local[e] == c)                (bf16 one-hot)
    PE :  psum[c,(j,i)] += S^T @ prod                 (bf16 MM, f32 PSUM)
    DVE:  off[c,i] = sum_j psum[c,(j,i)]              (strided reduce, f32)
    DVE:  T_next = a*(deg*x - off) - x [ *2 - T_prev ];  r += w_k*T_next
    CC :  AllGather(padded T_next slice) -> full padded table

Edge layout: per core, edges are bucketed by (dest 128-node group g, source
sub-table class t), each (g,t) segment padded to a block multiple of 128
(uniform across cores so the SPMD instruction stream is identical; per-core
variation lives in the data). Q is staged partition-major ([128, nblk*256]
bf16) so per-group loads are 128 contiguous runs of nbg*512B.
"""

import math
import os

import ml_dtypes
import numpy as np

import concourse.bacc as bacc
import concourse.bass as bass
import concourse.mybir as mybir
import concourse.tile as tile
from concourse.bass_utils import run_bass_kernel_spmd

N_CORES = 8
D = 16
ORDER = 4
P = 128
ES = 64  # padded table slot size (floats) = 256B, dma_gather elem granularity
N_CLS = 4  # sub-table classes (int16 index range)
SG = 6  # groups per gather super-group
PREP_GATHER = False  # prepare_only+trigger_dma gathers (False: blocking)
CHUNK_BLOCKS = 8  # 128-edge blocks per dma_gather call (16 hangs the SWDGE ring)

BF16 = ml_dtypes.bfloat16

# ---------------------------------------------------------------------------
# walrus workaround: this build rejects instructions with >1 semaphore wait
# ("Too many sync wait commands"). Spill overflow waits onto dedicated nops
# placed just before the offender on the same engine.
_spill_counter = [0]


def _spill_excess_waits(nc, max_waits=1):
    n_spilled = 0
    for fn in nc.m.functions:
        for bb in fn.blocks:
            insts = list(bb.instructions)
            new_list = []
            changed = False
            for inst in insts:
                si = inst.sync_info
                waits = list(si.on_wait) if si is not None and si.on_wait else []
                if len(waits) > max_waits:
                    changed = True
                    keep = waits[-max_waits:]
                    overflow = waits[: len(waits) - max_waits]
                    for i in range(0, len(overflow), max_waits):
                        chunk = overflow[i : i + max_waits]
                        _spill_counter[0] += 1
                        nop = mybir.InstNoOp(
                            name=f"waitspill-{_spill_counter[0]}",
                            engine=inst.engine,
                            bass_nofuse=True,
                            sync_info=mybir.SyncInfo(on_wait=chunk, on_update=[]),
                        )
                        new_list.append(nop)
                        n_spilled += 1
                    si.on_wait = keep
                new_list.append(inst)
            if changed:
                live = bb.instructions
                live.clear()
                live.extend(new_list)
    return n_spilled


# ---------------------------------------------------------------------------
# host preprocessing


def _table_idx(n, npc, n_groups):
    """node id -> slot in the permuted node table layout."""
    c = n // npc
    r = n % npc
    p = r % P
    g = r // P
    return (c * P + p) * n_groups + g


def _wrap_idx16(flat):
    """Wrap a flat int16 index stream into dma_gather's [128, n/16] layout
    (16-partition interleave, replicated 8x down the partitions)."""
    n = flat.shape[0]
    assert n % 16 == 0
    w = np.zeros((16, n // 16), dtype=np.int16)
    for p in range(16):
        w[p] = flat[p::16]
    return np.tile(w, (8, 1))


def _preprocess(h, Q, coeffs, edge_index, lambda_max):
    N = h.shape[0]
    E = edge_index.shape[1]
    assert N % N_CORES == 0
    npc = N // N_CORES
    n_groups = math.ceil(npc / P)
    slots = n_groups * P  # node slots per core
    table_rows = N_CORES * slots
    subt = (table_rows + N_CLS - 1) // N_CLS  # sub-table slot count
    assert subt <= 32767, "int16 sub-table index overflow"

    row = np.asarray(edge_index[0], dtype=np.int64)
    col = np.asarray(edge_index[1], dtype=np.int64)
    h = np.asarray(h, dtype=np.float32)
    Q = np.asarray(Q, dtype=np.float32)

    lam = float(np.asarray(lambda_max, dtype=np.float64))
    alpha = 2.0 / (lam + 1e-8)
    cf = np.asarray(coeffs, dtype=np.float64)
    w = np.exp(cf - cf.max())
    w = w / w.sum()

    degree = (
        np.bincount(row, minlength=N) + np.bincount(col, minlength=N)
    ).astype(np.float32)

    col_tab_all = _table_idx(col, npc, n_groups)
    cls_all = col_tab_all // subt  # gather class per edge

    # sort edges by (core, group, class) — host-side bucketing
    core_all = row // npc
    g_all = (row % npc) // P
    key = ((core_all * n_groups) + g_all) * N_CLS + cls_all
    perm = np.argsort(key, kind="stable")
    key_s = key[perm]
    row_s = row[perm]
    col_s = col[perm]
    col_tab = col_tab_all[perm]
    cls_s = cls_all[perm]

    n_seg = N_CORES * n_groups * N_CLS
    sizes = np.bincount(key_s, minlength=n_seg).reshape(N_CORES, n_groups * N_CLS)
    # uniform block count per (g,t) across cores
    NB_gt = ((sizes.max(axis=0) + P - 1) // P).astype(np.int64)
    NB_gt = NB_gt.reshape(n_groups, N_CLS)
    # each group needs >=1 block so its PSUM accumulation group exists
    empty_g = NB_gt.sum(axis=1) == 0
    NB_gt[empty_g, 0] = 1
    NB_g = NB_gt.sum(axis=1)  # [n_groups]
    gt_base = np.zeros(n_groups * N_CLS, dtype=np.int64)
    gt_base[1:] = np.cumsum(NB_gt.reshape(-1))[:-1]
    gt_base = gt_base.reshape(n_groups, N_CLS)
    g_base = np.zeros(n_groups + 1, dtype=np.int64)
    g_base[1:] = np.cumsum(NB_g)
    nblk = int(g_base[-1])

    # per-edge placement: rank within (core, g, t) segment
    seg_start = np.zeros(n_seg, dtype=np.int64)
    np.cumsum(sizes.reshape(-1)[:-1], out=seg_start[1:])
    rank = np.arange(E, dtype=np.int64) - seg_start[key_s]
    p_e = rank % P
    gt_of_edge = key_s % (n_groups * N_CLS)
    blk_e = gt_base.reshape(-1)[gt_of_edge] + rank // P

    rowl_e = ((row_s % npc) % P).astype(np.float32)

    # class-stream prefix: pos_t[t, g] = class-t blocks before group g
    pos_t = np.zeros((N_CLS, n_groups + 1), dtype=np.int64)
    for t in range(N_CLS):
        pos_t[t, 1:] = np.cumsum(NB_gt[:, t])
    nblk_t = pos_t[:, -1]

    deg_pad = np.zeros((N_CORES, slots), dtype=np.float32)
    deg_pad[:, :npc] = degree.reshape(N_CORES, npc)
    deg_t = np.ascontiguousarray(
        deg_pad.reshape(N_CORES, n_groups, P).transpose(0, 2, 1)
    )

    iota = np.tile(np.arange(P, dtype=np.float32), (P, 1)).astype(BF16)

    in_maps = []
    for c in range(N_CORES):
        m = (key_s // (n_groups * N_CLS)) == c
        e_idx = perm[m]
        p_c = p_e[m]
        blk_c = blk_e[m]
        t_c = cls_s[m]
        gt_c = gt_of_edge[m]
        g_c = gt_c // N_CLS

        # Q staged partition-major bf16, (i,j)-major within a block so the
        # prod multiply and the j-reduce stream at unit stride (DVE 2x mode)
        Qs = np.zeros((P, nblk, D * D), dtype=BF16)
        Qs[p_c, blk_c] = Q[e_idx].reshape(-1, D * D).astype(BF16)
        Qs = Qs.reshape(P, nblk * D * D)

        # step-1 gather precomputed on host: xc1[p, blk*D:(blk+1)*D] = h[col]
        xc1 = np.zeros((P, nblk, D), dtype=BF16)
        xc1[p_c, blk_c] = h[col_s[m]].astype(BF16)
        xc1 = xc1.reshape(P, nblk * D)

        rowl_t = np.zeros((P, nblk), dtype=BF16)
        rowl_t[p_c, blk_c] = rowl_e[m].astype(BF16)

        # per-class index streams (position = class-stream block * 128 + p)
        idx_cols = []
        for t in range(N_CLS):
            stream = np.zeros(max(int(nblk_t[t]), 0) * P, dtype=np.int16)
            mt = t_c == t
            gs = g_c[mt]
            pos_in_stream = (
                pos_t[t][gs] + (blk_c[mt] - gt_base[gs, t])
            ) * P + p_c[mt]
            stream[pos_in_stream] = (col_tab[m][mt] - t * subt).astype(np.int16)
            if stream.shape[0]:
                idx_cols.append(_wrap_idx16(stream))
        idx_arr = np.concatenate(idx_cols, axis=1)
        assert idx_arr.shape == (P, nblk * 8)

        x0 = np.zeros((P, n_groups * D), dtype=np.float32)
        x0r = x0.reshape(P, n_groups, D)
        hslice = h.reshape(N_CORES, npc, D)[c]
        full_g = npc // P
        x0r[:, :full_g] = (
            hslice[: full_g * P].reshape(full_g, P, D).transpose(1, 0, 2)
        )
        if npc % P:
            x0r[: npc % P, full_g] = hslice[full_g * P :]

        in_maps.append(
            {
                "qs": Qs,
                "xc1": xc1,
                "rowl_t": rowl_t,
                "idx_in": idx_arr,
                "deg_t": deg_t[c],
                "iota": iota,
                "x0": x0,
            }
        )

    meta = dict(
        N=N,
        npc=npc,
        n_groups=n_groups,
        slots=slots,
        table_rows=table_rows,
        subt=subt,
        nblk=nblk,
        NB_gt=NB_gt,
        NB_g=NB_g,
        gt_base=gt_base,
        g_base=g_base,
        pos_t=pos_t,
        nblk_t=nblk_t,
        alpha=alpha,
        w=[float(x) for x in w],
    )
    return in_maps, meta


# ---------------------------------------------------------------------------
# device program


def _build_nc(meta, detect_races=True):
    n_groups = meta["n_groups"]
    nblk = meta["nblk"]
    subt = meta["subt"]
    NB_gt = meta["NB_gt"]
    NB_g = meta["NB_g"]
    gt_base = meta["gt_base"]
    g_base = meta["g_base"]
    pos_t = meta["pos_t"]
    nblk_t = meta["nblk_t"]
    alpha = meta["alpha"]
    w = meta["w"]
    GF = n_groups * D
    max_nbg = int(NB_g.max())
    max_nbgt = int(NB_gt.max())
    n_sg = (n_groups + SG - 1) // SG
    max_sg_t = 1
    for sgi in range(n_sg):
        g0, g1 = sgi * SG, min((sgi + 1) * SG, n_groups)
        for t in range(N_CLS):
            max_sg_t = max(max_sg_t, int(pos_t[t, g1] - pos_t[t, g0]))

    # class offsets into idx_in (in columns; 8 columns per block)
    cls_col = np.zeros(N_CLS + 1, dtype=np.int64)
    cls_col[1:] = np.cumsum(nblk_t) * 8

    f32 = mybir.dt.float32
    bf16 = mybir.dt.bfloat16
    i16 = mybir.dt.int16
    nc = bacc.Bacc(None, detect_race_conditions=detect_races)

    qs = nc.declare_dram_parameter("qs", [P, nblk * D * D], bf16, isOutput=False)
    xc1_in = nc.declare_dram_parameter("xc1", [P, nblk * D], bf16, isOutput=False)
    rowl_in = nc.declare_dram_parameter("rowl_t", [P, nblk], bf16, isOutput=False)
    idx_in = nc.declare_dram_parameter("idx_in", [P, nblk * 8], i16, isOutput=False)
    deg_in = nc.declare_dram_parameter("deg_t", [P, n_groups], f32, isOutput=False)
    iota_in = nc.declare_dram_parameter("iota", [P, P], bf16, isOutput=False)
    x0_in = nc.declare_dram_parameter("x0", [P, GF], f32, isOutput=False)
    r_out = nc.declare_dram_parameter("r_out", [P, GF], f32, isOutput=True)

    ag_in = [
        nc.dram_tensor(f"ag_in{k}", [P * n_groups, ES], f32) for k in (1, 2, 3)
    ]
    ag_out = [
        nc.dram_tensor(f"ag_out{k}", [subt * N_CLS, ES], f32, addr_space="Shared")
        for k in (1, 2, 3)
    ]

    with tile.TileContext(nc) as tc:
        # NOTE: must be allocated inside TileContext — the context resets the
        # free-semaphore pool at entry, so an earlier allocation would be
        # handed out again to Tile's internal DMA lanes (races, NaNs).
        dma_sem = nc.alloc_semaphore("swdge_gather")
        with (
            tc.tile_pool(name="const", bufs=1) as const_pool,
            tc.tile_pool(name="state", bufs=1) as state_pool,
            tc.tile_pool(name="qg", bufs=3) as q_pool,
            tc.tile_pool(name="stage", bufs=2) as stage_pool,
            tc.tile_pool(name="idx", bufs=1) as idx_pool,
            tc.tile_pool(name="xcb", bufs=2) as xcb_pool,
            tc.tile_pool(name="prod", bufs=3) as prod_pool,
            tc.tile_pool(name="qh", bufs=3) as qh_pool,
            tc.tile_pool(name="sgen", bufs=3) as s_pool,
            tc.tile_pool(name="off", bufs=2) as off_pool,
            tc.tile_pool(name="canary", bufs=1) as canary_pool,
            tc.tile_pool(name="upd", bufs=2) as upd_pool,
            tc.tile_pool(name="psum", bufs=4, space="PSUM") as psum_pool,
        ):
            rowl_t = const_pool.tile([P, nblk], bf16)
            deg_t = const_pool.tile([P, n_groups], f32)
            iota_t = const_pool.tile([P, P], bf16)
            idx_t = idx_pool.tile([P, nblk * 8], i16)
            xcb1 = const_pool.tile([P, nblk * D], bf16)
            nc.sync.dma_start(out=rowl_t[:], in_=rowl_in[:])
            nc.sync.dma_start(out=deg_t[:], in_=deg_in[:])
            nc.sync.dma_start(out=iota_t[:], in_=iota_in[:])
            nc.sync.dma_start(out=idx_t[:], in_=idx_in[:])
            nc.sync.dma_start(out=xcb1[:], in_=xc1_in[:])

            xa = state_pool.tile([P, GF], f32, tag="xa")
            xb = state_pool.tile([P, GF], f32, tag="xb")
            r_t = state_pool.tile([P, GF], f32, tag="r")
            nc.sync.dma_start(out=xa[:], in_=x0_in[:])
            x, tp = xa, xb
            prep_count = [0]  # cumulative prepare_only gathers (16 incs each)

            for k in range(1, ORDER + 1):
                wk = w[k]
                cur_table = ag_out[k - 2] if k >= 2 else None
                off_full = off_pool.tile([P, GF], f32, tag="off")
                for sgi in range(n_sg):
                    g0, g1 = sgi * SG, min((sgi + 1) * SG, n_groups)
                    xcb_sg = [None] * N_CLS
                    for t in range(N_CLS):
                        nb = int(pos_t[t, g1] - pos_t[t, g0])
                        if nb == 0:
                            continue
                        if k == 1:
                            # step 1: host-precomputed gather (xcb1) is read
                            # directly by the prod op below; nothing to stage.
                            continue
                        xcb = xcb_pool.tile(
                            [P, max_sg_t * D], bf16, tag=f"xcb{t}"
                        )
                        st = stage_pool.tile(
                            [P, max_sg_t * ES], f32, tag=f"stage{t}"
                        )
                        for b0 in range(0, nb, CHUNK_BLOCKS):
                            bn = min(CHUNK_BLOCKS, nb - b0)
                            c0 = int(cls_col[t] + (pos_t[t, g0] + b0) * 8)
                            kw = (
                                dict(prepare_only=True, sem=dma_sem)
                                if PREP_GATHER
                                else {}
                            )
                            nc.gpsimd.dma_gather(
                                out_ap=st[
                                    :, b0 * ES : (b0 + bn) * ES
                                ].rearrange("p (b e) -> p b e", e=ES),
                                in_ap=cur_table[
                                    t * subt : (t + 1) * subt, :
                                ],
                                idxs_ap=idx_t[:, c0 : c0 + bn * 8],
                                num_idxs=bn * P,
                                num_idxs_reg=bn * P,
                                elem_size=ES,
                                **kw,
                            )
                            if PREP_GATHER:
                                prep_count[0] += 1
                        if PREP_GATHER:
                            nc.gpsimd.trigger_dma(count=None)
                        # cast on the (idle) scalar engine, freeing DVE
                        cast = nc.scalar.copy(
                            out=xcb[:, : nb * D].rearrange(
                                "p (b j) -> p b j", j=D
                            ),
                            in_=st[:, : nb * ES]
                            .rearrange("p (b e) -> p b e", e=ES)[:, :, :D],
                        )
                        if PREP_GATHER:
                            # manual RAW guard: Tile's deferred-write gating
                            # of prep-mode gather consumers is unreliable;
                            # enforce DMA completion on the reader directly.
                            cast._wait_ge(dma_sem, 16 * prep_count[0])
                        xcb_sg[t] = xcb
                    for g in range(g0, g1):
                        nbg = int(NB_g[g])
                        qg = q_pool.tile([P, max_nbg * D * D], bf16, tag="qg")
                        nc.sync.dma_start(
                            out=qg[:, : nbg * D * D],
                            in_=qs[
                                :,
                                int(g_base[g])
                                * D
                                * D : int(g_base[g + 1])
                                * D
                                * D,
                            ],
                        )
                        acc = psum_pool.tile([P, D], f32, tag="acc")
                        for t in range(N_CLS):
                            nb = int(NB_gt[g, t])
                            if nb == 0:
                                continue
                            segoff = int(gt_base[g, t] - g_base[g])
                            stoff = int(pos_t[t, g] - pos_t[t, g0])
                            if k == 1:
                                xc_ap = xcb1[
                                    :,
                                    int(gt_base[g, t])
                                    * D : (int(gt_base[g, t]) + nb)
                                    * D,
                                ]
                            else:
                                xc_ap = xcb_sg[t][
                                    :, stoff * D : (stoff + nb) * D
                                ]
                            prod = prod_pool.tile(
                                [P, max_nbgt * D * D], bf16, tag="prod"
                            )
                            nc.vector.tensor_tensor(
                                out=prod[:, : nb * D * D].rearrange(
                                    "p (b i j) -> p b i j", b=nb, i=D
                                ),
                                in0=qg[
                                    :, segoff * D * D : (segoff + nb) * D * D
                                ].rearrange("p (b i j) -> p b i j", b=nb, i=D),
                                in1=xc_ap
                                .rearrange("p (b j) -> p b j", j=D)
                                .unsqueeze(2)
                                .to_broadcast([P, nb, D, D]),
                                op=mybir.AluOpType.mult,
                            )
                            qh = qh_pool.tile(
                                [P, max_nbgt * D], bf16, tag="qh"
                            )
                            # bf16 j-accumulation: validated 2.5e-3 rel err
                            # at full scale vs the 2e-2 tolerance. j is the
                            # unit-stride inner axis (2x_2P eligible).
                            with nc.allow_low_precision(
                                reason="bf16 16-term j-sum, 8x under tol"
                            ):
                                nc.vector.tensor_reduce(
                                    out=qh[:, : nb * D].rearrange(
                                        "p (b i) -> p b i", i=D
                                    ),
                                    in_=prod[:, : nb * D * D].rearrange(
                                        "p (b i j) -> p b i j", b=nb, i=D
                                    ),
                                    axis=mybir.AxisListType.X,
                                    op=mybir.AluOpType.add,
                                )
                            S8 = s_pool.tile([P, max_nbgt * P], bf16, tag="S8")
                            nc.vector.tensor_tensor(
                                out=S8[:, : nb * P].rearrange(
                                    "p (b c) -> p b c", b=nb
                                ),
                                in0=rowl_t[
                                    :,
                                    int(gt_base[g, t]) : int(gt_base[g, t]) + nb,
                                ]
                                .unsqueeze(2)
                                .to_broadcast([P, nb, P]),
                                in1=iota_t[:]
                                .unsqueeze(1)
                                .to_broadcast([P, nb, P]),
                                op=mybir.AluOpType.is_equal,
                            )
                            for b in range(nb):
                                blk = int(gt_base[g, t]) + b
                                nc.tensor.matmul(
                                    out=acc[:],
                                    lhsT=S8[:, b * P : (b + 1) * P],
                                    rhs=qh[:, b * D : (b + 1) * D],
                                    start=(blk == int(g_base[g])),
                                    stop=(blk == int(g_base[g + 1]) - 1),
                                )
                        nc.scalar.copy(
                            out=off_full[:, g * D : (g + 1) * D],
                            in_=acc[:],
                        )

                # ---- node update (whole slice at once) ----
                tmp = upd_pool.tile([P, GF], f32, tag="tmp")
                nc.vector.tensor_tensor(
                    out=tmp[:].rearrange("p (g i) -> p g i", g=n_groups),
                    in0=x[:].rearrange("p (g i) -> p g i", g=n_groups),
                    in1=deg_t[:].unsqueeze(2).to_broadcast([P, n_groups, D]),
                    op=mybir.AluOpType.mult,
                )
                nc.vector.tensor_tensor(
                    out=tmp[:],
                    in0=tmp[:],
                    in1=off_full[:],
                    op=mybir.AluOpType.subtract,
                )
                nc.vector.tensor_scalar_mul(tmp[:], tmp[:], alpha)
                tmp2 = upd_pool.tile([P, GF], f32, tag="tmp2")
                if k == 1:
                    nc.vector.tensor_tensor(
                        out=tp[:], in0=tmp[:], in1=x[:], op=mybir.AluOpType.subtract
                    )
                    nc.vector.tensor_scalar_mul(r_t[:], x[:], w[0])
                    nc.vector.tensor_scalar_mul(tmp2[:], tp[:], wk)
                    nc.vector.tensor_tensor(
                        out=r_t[:], in0=r_t[:], in1=tmp2[:], op=mybir.AluOpType.add
                    )
                else:
                    nc.vector.tensor_tensor(
                        out=tmp[:], in0=tmp[:], in1=x[:], op=mybir.AluOpType.subtract
                    )
                    nc.vector.tensor_scalar_mul(tmp[:], tmp[:], 2.0)
                    nc.vector.tensor_tensor(
                        out=tp[:], in0=tmp[:], in1=tp[:], op=mybir.AluOpType.subtract
                    )
                    nc.vector.tensor_scalar_mul(tmp2[:], tp[:], wk)
                    nc.vector.tensor_tensor(
                        out=r_t[:], in0=r_t[:], in1=tmp2[:], op=mybir.AluOpType.add
                    )
                x, tp = tp, x  # x now holds T_k

                if k < ORDER:
                    dst = ag_in[k - 1]
                    nc.sync.dma_start(
                        out=dst[:].rearrange("(p g) e -> p g e", p=P)[:, :, :D],
                        in_=x[:].rearrange("p (g i) -> p g i", g=n_groups),
                    )
                    nc.gpsimd.collective_compute(
                        "AllGather",
                        mybir.AluOpType.bypass,
                        ins=[dst[:]],
                        outs=[ag_out[k - 1][:]],
                        replica_groups=[list(range(N_CORES))],
                    )
                    # Canary: blocking Pool-engine read of the AG output.
                    # Tile resolves gen-mode-0 collective->reader deps
                    # correctly (waits for ncfw completion); Pool in-order
                    # execution then fences the next step's trigger_dma
                    # calls behind AG completion. The deferred-dep path
                    # (trigger waiting on the collective directly) fires
                    # too early on HW.
                    canary = canary_pool.tile([P, 2], f32, tag="canary")
                    nc.gpsimd.dma_start(
                        out=canary[:], in_=ag_out[k - 1][0:P, 0:2]
                    )

            nc.sync.dma_start(out=r_out[:], in_=r_t[:])

    nc.compile()
    return nc


# ---------------------------------------------------------------------------

_CACHE = {}
LAST_RESULTS = None


def kernel(h, Q, coeffs, edge_index, lambda_max):
    h = np.asarray(h)
    Q = np.asarray(Q)
    coeffs = np.asarray(coeffs)
    edge_index = np.asarray(edge_index)
    lambda_max = np.asarray(lambda_max)

    import time as _time

    _t0 = _time.time()
    in_maps, meta = _preprocess(h, Q, coeffs, edge_index, lambda_max)
    _t1 = _time.time()

    key = (h.shape, Q.shape, edge_index.shape, meta["nblk"])
    if key not in _CACHE:
        nc = _build_nc(meta)
        _spill_excess_waits(nc)
        _CACHE[key] = nc
    nc = _CACHE[key]
    _t2 = _time.time()
    print(f"[kernel] preprocess {_t1-_t0:.1f}s  build+compile {_t2-_t1:.1f}s  nblk={meta['nblk']}", flush=True)

    trace = os.environ.get("CHEB_TRACE") == "1"
    res = run_bass_kernel_spmd(nc, in_maps, list(range(N_CORES)), trace=trace)
    print(f"[kernel] run {_time.time()-_t2:.1f}s", flush=True)
    global LAST_RESULTS
    LAST_RESULTS = res

    npc = meta["npc"]
    n_groups = meta["n_groups"]
    slots = meta["slots"]
    out = np.empty((h.shape[0], D), dtype=np.float32)
    for c in range(N_CORES):
        r = res.results[c]["r_out"]
        r3 = r.reshape(P, n_groups, D).transpose(1, 0, 2).reshape(slots, D)
        out[c * npc : (c + 1) * npc] = r3[:npc]
    return out


# revision 36
# speedup vs baseline: 3.1421x; 1.0131x over previous
"""ChebyshevSheafFilter Trainium2 kernel (8 NeuronCores, edge/row-parallel).

Self-contained: host-side preprocessing (numpy) + Bass/Tile SPMD kernel via
run_bass_kernel_spmd. Takes FULL inputs, returns FULL output.

Math (matches reference):
    degree = bincount(row) + bincount(col)
    L(x)   = (2/(lam+1e-8)) * (degree*x - segsum_row(Q[e] @ x[col[e]])) - x
    T0=h, T1=L(h), Tk = 2 L(T_{k-1}) - T_{k-2}
    out = sum_k softmax(coeffs)[k] * Tk

Sharding: edges sorted by destination row; core c owns node rows
[c*npc,(c+1)*npc). Per Chebyshev step each core computes its own node slice
of T_next, then an 8-core AllGather rebuilds the full node table (gather
source for the next step's x[col]).

Device pipeline per step (per core):
    GPS dma_gather (prepare_only + trigger): xc = x[col] from a 256B-padded
        node table (4 int16-indexed sub-tables); step 1 instead reads a
        host-precomputed edge-aligned xc1 (bf16, no gather needed)
    DVE:  xcb = bf16(xc[.., :16])                     (cast for 2x matmul)
    DVE:  prod[e,(j,i)] = Qb[e,(j,i)] * xcb[e,j]      (bf16, stride-0 bcast)
    DVE:  S[e,c] = (row_local[e] == c)                (bf16 one-hot)
    PE :  psum[c,(j,i)] += S^T @ prod                 (bf16 MM, f32 PSUM)
    DVE:  off[c,i] = sum_j psum[c,(j,i)]              (strided reduce, f32)
    DVE:  T_next = a*(deg*x - off) - x [ *2 - T_prev ];  r += w_k*T_next
    CC :  AllGather(padded T_next slice) -> full padded table

Edge layout: per core, edges are bucketed by (dest 128-node group g, source
sub-table class t), each (g,t) segment padded to a block multiple of 128
(uniform across cores so the SPMD instruction stream is identical; per-core
variation lives in the data). Q is staged partition-major ([128, nblk*256]
bf16) so per-group loads are 128 contiguous runs of nbg*512B.
"""

import math
import os

import ml_dtypes
import numpy as np

import concourse.bacc as bacc
import concourse.bass as bass
import concourse.mybir as mybir
import concourse.tile as tile
from concourse.bass_utils import run_bass_kernel_spmd

N_CORES = 8
D = 16
ORDER = 4
P = 128
ES = 64  # padded table slot size (floats) = 256B, dma_gather elem granularity
N_CLS = 4  # sub-table classes (int16 index range)
SG = 12  # groups per gather super-group
PREP_GATHER = False  # prepare_only+trigger_dma gathers (False: blocking)
CHUNK_BLOCKS = 8  # 128-edge blocks per dma_gather call (16 hangs the SWDGE ring)

BF16 = ml_dtypes.bfloat16

# ---------------------------------------------------------------------------
# walrus workaround: this build rejects instructions with >1 semaphore wait
# ("Too many sync wait commands"). Spill overflow waits onto dedicated nops
# placed just before the offender on the same engine.
_spill_counter = [0]


def _spill_excess_waits(nc, max_waits=1):
    n_spilled = 0
    for fn in nc.m.functions:
        for bb in fn.blocks:
            insts = list(bb.instructions)
            new_list = []
            changed = False
            for inst in insts:
                si = inst.sync_info
                waits = list(si.on_wait) if si is not None and si.on_wait else []
                if len(waits) > max_waits:
                    changed = True
                    keep = waits[-max_waits:]
                    overflow = waits[: len(waits) - max_waits]
                    for i in range(0, len(overflow), max_waits):
                        chunk = overflow[i : i + max_waits]
                        _spill_counter[0] += 1
                        nop = mybir.InstNoOp(
                            name=f"waitspill-{_spill_counter[0]}",
                            engine=inst.engine,
                            bass_nofuse=True,
                            sync_info=mybir.SyncInfo(on_wait=chunk, on_update=[]),
                        )
                        new_list.append(nop)
                        n_spilled += 1
                    si.on_wait = keep
                new_list.append(inst)
            if changed:
                live = bb.instructions
                live.clear()
                live.extend(new_list)
    return n_spilled


# ---------------------------------------------------------------------------
# host preprocessing


def _table_idx(n, npc, n_groups):
    """node id -> slot in the permuted node table layout."""
    c = n // npc
    r = n % npc
    p = r % P
    g = r // P
    return (c * P + p) * n_groups + g


def _wrap_idx16(flat):
    """Wrap a flat int16 index stream into dma_gather's [128, n/16] layout
    (16-partition interleave, replicated 8x down the partitions)."""
    n = flat.shape[0]
    assert n % 16 == 0
    w = np.zeros((16, n // 16), dtype=np.int16)
    for p in range(16):
        w[p] = flat[p::16]
    return np.tile(w, (8, 1))


def _preprocess(h, Q, coeffs, edge_index, lambda_max):
    N = h.shape[0]
    E = edge_index.shape[1]
    assert N % N_CORES == 0
    npc = N // N_CORES
    n_groups = math.ceil(npc / P)
    slots = n_groups * P  # node slots per core
    table_rows = N_CORES * slots
    subt = (table_rows + N_CLS - 1) // N_CLS  # sub-table slot count
    assert subt <= 32767, "int16 sub-table index overflow"

    row = np.asarray(edge_index[0], dtype=np.int64)
    col = np.asarray(edge_index[1], dtype=np.int64)
    h = np.asarray(h, dtype=np.float32)
    Q = np.asarray(Q, dtype=np.float32)

    lam = float(np.asarray(lambda_max, dtype=np.float64))
    alpha = 2.0 / (lam + 1e-8)
    cf = np.asarray(coeffs, dtype=np.float64)
    w = np.exp(cf - cf.max())
    w = w / w.sum()

    degree = (
        np.bincount(row, minlength=N) + np.bincount(col, minlength=N)
    ).astype(np.float32)

    col_tab_all = _table_idx(col, npc, n_groups)
    cls_all = col_tab_all // subt  # gather class per edge

    # sort edges by (core, group, class) — host-side bucketing
    core_all = row // npc
    g_all = (row % npc) // P
    key = ((core_all * n_groups) + g_all) * N_CLS + cls_all
    perm = np.argsort(key, kind="stable")
    key_s = key[perm]
    row_s = row[perm]
    col_s = col[perm]
    col_tab = col_tab_all[perm]
    cls_s = cls_all[perm]

    n_seg = N_CORES * n_groups * N_CLS
    sizes = np.bincount(key_s, minlength=n_seg).reshape(N_CORES, n_groups * N_CLS)
    # uniform block count per (g,t) across cores
    NB_gt = ((sizes.max(axis=0) + P - 1) // P).astype(np.int64)
    NB_gt = NB_gt.reshape(n_groups, N_CLS)
    # each group needs >=1 block so its PSUM accumulation group exists
    empty_g = NB_gt.sum(axis=1) == 0
    NB_gt[empty_g, 0] = 1
    NB_g = NB_gt.sum(axis=1)  # [n_groups]
    gt_base = np.zeros(n_groups * N_CLS, dtype=np.int64)
    gt_base[1:] = np.cumsum(NB_gt.reshape(-1))[:-1]
    gt_base = gt_base.reshape(n_groups, N_CLS)
    g_base = np.zeros(n_groups + 1, dtype=np.int64)
    g_base[1:] = np.cumsum(NB_g)
    nblk = int(g_base[-1])

    # per-edge placement: rank within (core, g, t) segment
    seg_start = np.zeros(n_seg, dtype=np.int64)
    np.cumsum(sizes.reshape(-1)[:-1], out=seg_start[1:])
    rank = np.arange(E, dtype=np.int64) - seg_start[key_s]
    p_e = rank % P
    gt_of_edge = key_s % (n_groups * N_CLS)
    blk_e = gt_base.reshape(-1)[gt_of_edge] + rank // P

    rowl_e = ((row_s % npc) % P).astype(np.float32)

    # class-stream prefix: pos_t[t, g] = class-t blocks before group g
    pos_t = np.zeros((N_CLS, n_groups + 1), dtype=np.int64)
    for t in range(N_CLS):
        pos_t[t, 1:] = np.cumsum(NB_gt[:, t])
    nblk_t = pos_t[:, -1]

    deg_pad = np.zeros((N_CORES, slots), dtype=np.float32)
    deg_pad[:, :npc] = degree.reshape(N_CORES, npc)
    deg_t = np.ascontiguousarray(
        deg_pad.reshape(N_CORES, n_groups, P).transpose(0, 2, 1)
    )

    iota = np.tile(np.arange(P, dtype=np.float32), (P, 1)).astype(BF16)

    in_maps = []
    for c in range(N_CORES):
        m = (key_s // (n_groups * N_CLS)) == c
        e_idx = perm[m]
        p_c = p_e[m]
        blk_c = blk_e[m]
        t_c = cls_s[m]
        gt_c = gt_of_edge[m]
        g_c = gt_c // N_CLS

        # Q staged partition-major bf16, (i,j)-major within a block so the
        # prod multiply and the j-reduce stream at unit stride (DVE 2x mode)
        Qs = np.zeros((P, nblk, D * D), dtype=BF16)
        Qs[p_c, blk_c] = Q[e_idx].reshape(-1, D * D).astype(BF16)
        Qs = Qs.reshape(P, nblk * D * D)

        # step-1 gather precomputed on host: xc1[p, blk*D:(blk+1)*D] = h[col]
        xc1 = np.zeros((P, nblk, D), dtype=BF16)
        xc1[p_c, blk_c] = h[col_s[m]].astype(BF16)
        xc1 = xc1.reshape(P, nblk * D)

        rowl_t = np.zeros((P, nblk), dtype=BF16)
        rowl_t[p_c, blk_c] = rowl_e[m].astype(BF16)

        # per-class index streams (position = class-stream block * 128 + p)
        idx_cols = []
        for t in range(N_CLS):
            stream = np.zeros(max(int(nblk_t[t]), 0) * P, dtype=np.int16)
            mt = t_c == t
            gs = g_c[mt]
            pos_in_stream = (
                pos_t[t][gs] + (blk_c[mt] - gt_base[gs, t])
            ) * P + p_c[mt]
            stream[pos_in_stream] = (col_tab[m][mt] - t * subt).astype(np.int16)
            if stream.shape[0]:
                idx_cols.append(_wrap_idx16(stream))
        idx_arr = np.concatenate(idx_cols, axis=1)
        assert idx_arr.shape == (P, nblk * 8)

        x0 = np.zeros((P, n_groups * D), dtype=np.float32)
        x0r = x0.reshape(P, n_groups, D)
        hslice = h.reshape(N_CORES, npc, D)[c]
        full_g = npc // P
        x0r[:, :full_g] = (
            hslice[: full_g * P].reshape(full_g, P, D).transpose(1, 0, 2)
        )
        if npc % P:
            x0r[: npc % P, full_g] = hslice[full_g * P :]

        in_maps.append(
            {
                "qs": Qs,
                "xc1": xc1,
                "rowl_t": rowl_t,
                "idx_in": idx_arr,
                "deg_t": deg_t[c],
                "iota": iota,
                "x0": x0,
            }
        )

    meta = dict(
        N=N,
        npc=npc,
        n_groups=n_groups,
        slots=slots,
        table_rows=table_rows,
        subt=subt,
        nblk=nblk,
        NB_gt=NB_gt,
        NB_g=NB_g,
        gt_base=gt_base,
        g_base=g_base,
        pos_t=pos_t,
        nblk_t=nblk_t,
        alpha=alpha,
        w=[float(x) for x in w],
    )
    return in_maps, meta


# ---------------------------------------------------------------------------
# device program


def _build_nc(meta, detect_races=True):
    n_groups = meta["n_groups"]
    nblk = meta["nblk"]
    subt = meta["subt"]
    NB_gt = meta["NB_gt"]
    NB_g = meta["NB_g"]
    gt_base = meta["gt_base"]
    g_base = meta["g_base"]
    pos_t = meta["pos_t"]
    nblk_t = meta["nblk_t"]
    alpha = meta["alpha"]
    w = meta["w"]
    GF = n_groups * D
    max_nbg = int(NB_g.max())
    max_nbgt = int(NB_gt.max())
    n_sg = (n_groups + SG - 1) // SG
    max_sg_t = 1
    for sgi in range(n_sg):
        g0, g1 = sgi * SG, min((sgi + 1) * SG, n_groups)
        for t in range(N_CLS):
            max_sg_t = max(max_sg_t, int(pos_t[t, g1] - pos_t[t, g0]))

    # class offsets into idx_in (in columns; 8 columns per block)
    cls_col = np.zeros(N_CLS + 1, dtype=np.int64)
    cls_col[1:] = np.cumsum(nblk_t) * 8

    f32 = mybir.dt.float32
    bf16 = mybir.dt.bfloat16
    i16 = mybir.dt.int16
    nc = bacc.Bacc(None, detect_race_conditions=detect_races)

    qs = nc.declare_dram_parameter("qs", [P, nblk * D * D], bf16, isOutput=False)
    xc1_in = nc.declare_dram_parameter("xc1", [P, nblk * D], bf16, isOutput=False)
    rowl_in = nc.declare_dram_parameter("rowl_t", [P, nblk], bf16, isOutput=False)
    idx_in = nc.declare_dram_parameter("idx_in", [P, nblk * 8], i16, isOutput=False)
    deg_in = nc.declare_dram_parameter("deg_t", [P, n_groups], f32, isOutput=False)
    iota_in = nc.declare_dram_parameter("iota", [P, P], bf16, isOutput=False)
    x0_in = nc.declare_dram_parameter("x0", [P, GF], f32, isOutput=False)
    r_out = nc.declare_dram_parameter("r_out", [P, GF], f32, isOutput=True)

    ag_in = [
        nc.dram_tensor(f"ag_in{k}", [P * n_groups, ES], f32) for k in (1, 2, 3)
    ]
    ag_out = [
        nc.dram_tensor(f"ag_out{k}", [subt * N_CLS, ES], f32, addr_space="Shared")
        for k in (1, 2, 3)
    ]

    with tile.TileContext(nc) as tc:
        # NOTE: must be allocated inside TileContext — the context resets the
        # free-semaphore pool at entry, so an earlier allocation would be
        # handed out again to Tile's internal DMA lanes (races, NaNs).
        dma_sem = nc.alloc_semaphore("swdge_gather")
        with (
            tc.tile_pool(name="const", bufs=1) as const_pool,
            tc.tile_pool(name="state", bufs=1) as state_pool,
            tc.tile_pool(name="qg", bufs=3) as q_pool,
            tc.tile_pool(name="stage", bufs=2) as stage_pool,
            tc.tile_pool(name="idx", bufs=1) as idx_pool,
            tc.tile_pool(name="xcb", bufs=2) as xcb_pool,
            tc.tile_pool(name="prod", bufs=3) as prod_pool,
            tc.tile_pool(name="qh", bufs=3) as qh_pool,
            tc.tile_pool(name="sgen", bufs=3) as s_pool,
            tc.tile_pool(name="off", bufs=2) as off_pool,
            tc.tile_pool(name="canary", bufs=1) as canary_pool,
            tc.tile_pool(name="upd", bufs=1) as upd_pool,
            tc.tile_pool(name="psum", bufs=4, space="PSUM") as psum_pool,
        ):
            rowl_t = const_pool.tile([P, nblk], bf16)
            deg_t = const_pool.tile([P, n_groups], f32)
            iota_t = const_pool.tile([P, P], bf16)
            idx_t = idx_pool.tile([P, nblk * 8], i16)
            xcb1 = const_pool.tile([P, nblk * D], bf16)
            nc.sync.dma_start(out=rowl_t[:], in_=rowl_in[:])
            nc.sync.dma_start(out=deg_t[:], in_=deg_in[:])
            nc.sync.dma_start(out=iota_t[:], in_=iota_in[:])
            nc.sync.dma_start(out=idx_t[:], in_=idx_in[:])
            nc.sync.dma_start(out=xcb1[:], in_=xc1_in[:])

            xa = state_pool.tile([P, GF], f32, tag="xa")
            xb = state_pool.tile([P, GF], f32, tag="xb")
            r_t = state_pool.tile([P, GF], f32, tag="r")
            nc.sync.dma_start(out=xa[:], in_=x0_in[:])
            x, tp = xa, xb
            prep_count = [0]  # cumulative prepare_only gathers (16 incs each)

            for k in range(1, ORDER + 1):
                wk = w[k]
                cur_table = ag_out[k - 2] if k >= 2 else None
                off_full = off_pool.tile([P, GF], f32, tag="off")
                for sgi in range(n_sg):
                    g0, g1 = sgi * SG, min((sgi + 1) * SG, n_groups)
                    xcb_sg = [None] * N_CLS
                    for t in range(N_CLS):
                        nb = int(pos_t[t, g1] - pos_t[t, g0])
                        if nb == 0:
                            continue
                        if k == 1:
                            # step 1: host-precomputed gather (xcb1) is read
                            # directly by the prod op below; nothing to stage.
                            continue
                        xcb = xcb_pool.tile(
                            [P, max_sg_t * D], bf16, tag=f"xcb{t}"
                        )
                        st = stage_pool.tile(
                            [P, max_sg_t * ES], f32, tag=f"stage{t}"
                        )
                        for b0 in range(0, nb, CHUNK_BLOCKS):
                            bn = min(CHUNK_BLOCKS, nb - b0)
                            c0 = int(cls_col[t] + (pos_t[t, g0] + b0) * 8)
                            kw = (
                                dict(prepare_only=True, sem=dma_sem)
                                if PREP_GATHER
                                else {}
                            )
                            nc.gpsimd.dma_gather(
                                out_ap=st[
                                    :, b0 * ES : (b0 + bn) * ES
                                ].rearrange("p (b e) -> p b e", e=ES),
                                in_ap=cur_table[
                                    t * subt : (t + 1) * subt, :
                                ],
                                idxs_ap=idx_t[:, c0 : c0 + bn * 8],
                                num_idxs=bn * P,
                                num_idxs_reg=bn * P,
                                elem_size=ES,
                                **kw,
                            )
                            if PREP_GATHER:
                                prep_count[0] += 1
                        if PREP_GATHER:
                            nc.gpsimd.trigger_dma(count=None)
                        # cast on the (idle) scalar engine, freeing DVE
                        cast = nc.scalar.copy(
                            out=xcb[:, : nb * D].rearrange(
                                "p (b j) -> p b j", j=D
                            ),
                            in_=st[:, : nb * ES]
                            .rearrange("p (b e) -> p b e", e=ES)[:, :, :D],
                        )
                        if PREP_GATHER:
                            # manual RAW guard: Tile's deferred-write gating
                            # of prep-mode gather consumers is unreliable;
                            # enforce DMA completion on the reader directly.
                            cast._wait_ge(dma_sem, 16 * prep_count[0])
                        xcb_sg[t] = xcb
                    for g in range(g0, g1):
                        nbg = int(NB_g[g])
                        qg = q_pool.tile([P, max_nbg * D * D], bf16, tag="qg")
                        nc.sync.dma_start(
                            out=qg[:, : nbg * D * D],
                            in_=qs[
                                :,
                                int(g_base[g])
                                * D
                                * D : int(g_base[g + 1])
                                * D
                                * D,
                            ],
                        )
                        acc = psum_pool.tile([P, D], f32, tag="acc")
                        for t in range(N_CLS):
                            nb = int(NB_gt[g, t])
                            if nb == 0:
                                continue
                            segoff = int(gt_base[g, t] - g_base[g])
                            stoff = int(pos_t[t, g] - pos_t[t, g0])
                            if k == 1:
                                xc_ap = xcb1[
                                    :,
                                    int(gt_base[g, t])
                                    * D : (int(gt_base[g, t]) + nb)
                                    * D,
                                ]
                            else:
                                xc_ap = xcb_sg[t][
                                    :, stoff * D : (stoff + nb) * D
                                ]
                            prod = prod_pool.tile(
                                [P, max_nbgt * D * D], bf16, tag="prod"
                            )
                            nc.vector.tensor_tensor(
                                out=prod[:, : nb * D * D].rearrange(
                                    "p (b i j) -> p b i j", b=nb, i=D
                                ),
                                in0=qg[
                                    :, segoff * D * D : (segoff + nb) * D * D
                                ].rearrange("p (b i j) -> p b i j", b=nb, i=D),
                                in1=xc_ap
                                .rearrange("p (b j) -> p b j", j=D)
                                .unsqueeze(2)
                                .to_broadcast([P, nb, D, D]),
                                op=mybir.AluOpType.mult,
                            )
                            qh = qh_pool.tile(
                                [P, max_nbgt * D], bf16, tag="qh"
                            )
                            # bf16 j-accumulation: validated 2.5e-3 rel err
                            # at full scale vs the 2e-2 tolerance. j is the
                            # unit-stride inner axis (2x_2P eligible).
                            with nc.allow_low_precision(
                                reason="bf16 16-term j-sum, 8x under tol"
                            ):
                                nc.vector.tensor_reduce(
                                    out=qh[:, : nb * D].rearrange(
                                        "p (b i) -> p b i", i=D
                                    ),
                                    in_=prod[:, : nb * D * D].rearrange(
                                        "p (b i j) -> p b i j", b=nb, i=D
                                    ),
                                    axis=mybir.AxisListType.X,
                                    op=mybir.AluOpType.add,
                                )
                            S8 = s_pool.tile([P, max_nbgt * P], bf16, tag="S8")
                            nc.vector.tensor_tensor(
                                out=S8[:, : nb * P].rearrange(
                                    "p (b c) -> p b c", b=nb
                                ),
                                in0=rowl_t[
                                    :,
                                    int(gt_base[g, t]) : int(gt_base[g, t]) + nb,
                                ]
                                .unsqueeze(2)
                                .to_broadcast([P, nb, P]),
                                in1=iota_t[:]
                                .unsqueeze(1)
                                .to_broadcast([P, nb, P]),
                                op=mybir.AluOpType.is_equal,
                            )
                            for b in range(nb):
                                blk = int(gt_base[g, t]) + b
                                nc.tensor.matmul(
                                    out=acc[:],
                                    lhsT=S8[:, b * P : (b + 1) * P],
                                    rhs=qh[:, b * D : (b + 1) * D],
                                    start=(blk == int(g_base[g])),
                                    stop=(blk == int(g_base[g + 1]) - 1),
                                )
                        nc.scalar.copy(
                            out=off_full[:, g * D : (g + 1) * D],
                            in_=acc[:],
                        )

                # ---- node update (whole slice at once) ----
                tmp = upd_pool.tile([P, GF], f32, tag="tmp")
                nc.vector.tensor_tensor(
                    out=tmp[:].rearrange("p (g i) -> p g i", g=n_groups),
                    in0=x[:].rearrange("p (g i) -> p g i", g=n_groups),
                    in1=deg_t[:].unsqueeze(2).to_broadcast([P, n_groups, D]),
                    op=mybir.AluOpType.mult,
                )
                nc.vector.tensor_tensor(
                    out=tmp[:],
                    in0=tmp[:],
                    in1=off_full[:],
                    op=mybir.AluOpType.subtract,
                )
                nc.vector.tensor_scalar_mul(tmp[:], tmp[:], alpha)
                tmp2 = upd_pool.tile([P, GF], f32, tag="tmp2")
                if k == 1:
                    nc.vector.tensor_tensor(
                        out=tp[:], in0=tmp[:], in1=x[:], op=mybir.AluOpType.subtract
                    )
                    nc.vector.tensor_scalar_mul(r_t[:], x[:], w[0])
                    nc.vector.tensor_scalar_mul(tmp2[:], tp[:], wk)
                    nc.vector.tensor_tensor(
                        out=r_t[:], in0=r_t[:], in1=tmp2[:], op=mybir.AluOpType.add
                    )
                else:
                    nc.vector.tensor_tensor(
                        out=tmp[:], in0=tmp[:], in1=x[:], op=mybir.AluOpType.subtract
                    )
                    nc.vector.tensor_scalar_mul(tmp[:], tmp[:], 2.0)
                    nc.vector.tensor_tensor(
                        out=tp[:], in0=tmp[:], in1=tp[:], op=mybir.AluOpType.subtract
                    )
                    nc.vector.tensor_scalar_mul(tmp2[:], tp[:], wk)
                    nc.vector.tensor_tensor(
                        out=r_t[:], in0=r_t[:], in1=tmp2[:], op=mybir.AluOpType.add
                    )
                x, tp = tp, x  # x now holds T_k

                if k < ORDER:
                    dst = ag_in[k - 1]
                    nc.sync.dma_start(
                        out=dst[:].rearrange("(p g) e -> p g e", p=P)[:, :, :D],
                        in_=x[:].rearrange("p (g i) -> p g i", g=n_groups),
                    )
                    nc.gpsimd.collective_compute(
                        "AllGather",
                        mybir.AluOpType.bypass,
                        ins=[dst[:]],
                        outs=[ag_out[k - 1][:]],
                        replica_groups=[list(range(N_CORES))],
                    )
                    # Canary: blocking Pool-engine read of the AG output.
                    # Tile resolves gen-mode-0 collective->reader deps
                    # correctly (waits for ncfw completion); Pool in-order
                    # execution then fences the next step's trigger_dma
                    # calls behind AG completion. The deferred-dep path
                    # (trigger waiting on the collective directly) fires
                    # too early on HW.
                    canary = canary_pool.tile([P, 2], f32, tag="canary")
                    nc.gpsimd.dma_start(
                        out=canary[:], in_=ag_out[k - 1][0:P, 0:2]
                    )

            nc.sync.dma_start(out=r_out[:], in_=r_t[:])

    nc.compile()
    return nc


# ---------------------------------------------------------------------------

_CACHE = {}
LAST_RESULTS = None


def kernel(h, Q, coeffs, edge_index, lambda_max):
    h = np.asarray(h)
    Q = np.asarray(Q)
    coeffs = np.asarray(coeffs)
    edge_index = np.asarray(edge_index)
    lambda_max = np.asarray(lambda_max)

    import time as _time

    _t0 = _time.time()
    in_maps, meta = _preprocess(h, Q, coeffs, edge_index, lambda_max)
    _t1 = _time.time()

    key = (h.shape, Q.shape, edge_index.shape, meta["nblk"])
    if key not in _CACHE:
        nc = _build_nc(meta)
        _spill_excess_waits(nc)
        _CACHE[key] = nc
    nc = _CACHE[key]
    _t2 = _time.time()
    print(f"[kernel] preprocess {_t1-_t0:.1f}s  build+compile {_t2-_t1:.1f}s  nblk={meta['nblk']}", flush=True)

    trace = os.environ.get("CHEB_TRACE") == "1"
    res = run_bass_kernel_spmd(nc, in_maps, list(range(N_CORES)), trace=trace)
    print(f"[kernel] run {_time.time()-_t2:.1f}s", flush=True)
    global LAST_RESULTS
    LAST_RESULTS = res

    npc = meta["npc"]
    n_groups = meta["n_groups"]
    slots = meta["slots"]
    out = np.empty((h.shape[0], D), dtype=np.float32)
    for c in range(N_CORES):
        r = res.results[c]["r_out"]
        r3 = r.reshape(P, n_groups, D).transpose(1, 0, 2).reshape(slots, D)
        out[c * npc : (c + 1) * npc] = r3[:npc]
    return out
